# revision 25
# baseline (speedup 1.0000x reference)
"""BiLSTM+CRF NLL loss kernel for 8 Trainium2 NeuronCores (v3).

Sharding: data-parallel on batch (32 sequences per core). Each core runs the
full BiLSTM + emission + CRF forward/backward partition recurrences for its
shard; host combines per-core partials into the scalar loss.

v3 vs v2 (trace: 2551ns/step chain = mm,σ,tanh_g,mul,add,tanh_c,mul_h):
- tanh(g) removed from the serial ACT chain: g pre-acts are scaled 2x on the
  host so one sigmoid instruction covers f,i,g (tanh(g) = 2σ(2g)-1), and the
  cell update becomes c = f*c + 2(σ_g-0.5)*i via fused scalar_tensor_tensor
- cell/gate DVE pipeline in bf16 (2x DVE rate), cell updated in place in a
  fixed tile (no cross-engine WAR)
- CRF fwd/bwd hop multiplies merged into one strided-AP tensor_tensor
"""

import numpy as np
import ml_dtypes

import concourse.bass as bass
import concourse.tile as tile
from concourse import mybir
from concourse.bass_utils import run_bass_kernel_spmd

F32 = mybir.dt.float32
BF16 = mybir.dt.bfloat16

B, S, V, I, NB = 256, 512, 30000, 100, 19
BOS, EOS = 17, 18
NCORES = 8
BC = B // NCORES          # 32 sequences per core
NT = BC * S               # 16384 tokens per core
KP = I + 1                # 101: embedding dims + ones row (bias aug)
EPAD = 128                # padded embedding row length
RENORM = 16               # CRF renorm interval
TBLK = 4                  # steps per PSUM gate block
GCH = 8                   # gate chunks: (gamma in [g,f,i,o]) x (dir in [f,b])

_CACHE = {}


def _build_nc(s_len=S):
    SL = s_len
    NTL = BC * SL
    NBLK = SL // TBLK
    NCHUNK = NTL // 128

    nc = bass.Bass()

    # ---- dram I/O ----
    emb_d = nc.dram_tensor("emb_pad", [V, EPAD], BF16, kind="ExternalInput")
    idx_d = nc.dram_tensor("idxs", [128, NTL // 128], mybir.dt.int32, kind="ExternalInput")
    eye_d = nc.dram_tensor("eye", [128, 128], BF16, kind="ExternalInput")
    wih_d = nc.dram_tensor("wih", [128, GCH, 128], BF16, kind="ExternalInput")
    whh_d = nc.dram_tensor("whh", [128, GCH, 128], BF16, kind="ExternalInput")
    wc_d = nc.dram_tensor("wc", [128, 2, NB], BF16, kind="ExternalInput")
    bc_d = nc.dram_tensor("bc", [NB, 1], F32, kind="ExternalInput")
    esm_d = nc.dram_tensor("esm", [NB, NB], BF16, kind="ExternalInput")
    est_d = nc.dram_tensor("est", [NB, NB], BF16, kind="ExternalInput")
    etb_d = nc.dram_tensor("etb", [NB, 1], F32, kind="ExternalInput")
    veb_d = nc.dram_tensor("veb", [NB, BC], F32, kind="ExternalInput")
    ones19_d = nc.dram_tensor("ones19", [NB, 1], BF16, kind="ExternalInput")
    ones19f_d = nc.dram_tensor("ones19f", [NB, 1], F32, kind="ExternalInput")
    one1x19_d = nc.dram_tensor("one1x19", [1, NB], BF16, kind="ExternalInput")

    y_out = nc.dram_tensor("y_out", [NB, NTL], BF16, kind="ExternalOutput")
    res_out = nc.dram_tensor("res", [4, BC], F32, kind="ExternalOutput")

    SIG = mybir.ActivationFunctionType.Sigmoid
    TANH = mybir.ActivationFunctionType.Tanh
    EXP = mybir.ActivationFunctionType.Exp
    LOG = mybir.ActivationFunctionType.Ln

    with tile.TileContext(nc) as tc:
        with tc.tile_pool(name="big", bufs=1) as bp:
            xeT_f = bp.tile([128, NTL], BF16, tag="xeT_f")
            emstore = bp.tile([NB, NTL], BF16, tag="emstore")
            eye_s = bp.tile([128, 128], BF16, tag="eye_s")
            # h storage: col (t+1)*32 = h after step t; col 0 = h(-1)=0
            h_all = bp.tile([128, 2, NTL + BC], BF16, tag="h_all")
            Y = bp.tile([NB, NTL], BF16, tag="Y")
            idx_f = bp.tile([128, NTL // 128], mybir.dt.int32, tag="idx_f")
            wih = bp.tile([128, GCH, 128], BF16, tag="wih")
            whh = bp.tile([128, GCH, 128], BF16, tag="whh")
            wc = bp.tile([128, 2, NB], BF16, tag="wc")
            bc_s = bp.tile([NB, 1], F32, tag="bc_s")
            esm = bp.tile([NB, NB], BF16, tag="esm")
            est = bp.tile([NB, NB], BF16, tag="est")
            etb = bp.tile([NB, 1], F32, tag="etb")
            veb = bp.tile([NB, BC], F32, tag="veb")
            ones19 = bp.tile([NB, 1], BF16, tag="ones19")
            ones19f = bp.tile([NB, 1], F32, tag="ones19f")
            one1x19 = bp.tile([1, NB], BF16, tag="one1x19")
            gates_s0 = bp.tile([128, GCH, BC], BF16, tag="gates_s0")
            gates_s1 = bp.tile([128, GCH, BC], BF16, tag="gates_s1")
            cellc = bp.tile([128, 2, BC], BF16, tag="cellc")  # c, in-place
            th0 = bp.tile([128, 2, BC], BF16, tag="th0")
            th1 = bp.tile([128, 2, BC], BF16, tag="th1")
            u_f = bp.tile([128, 2, BC], BF16, tag="u_f")
            u_i = bp.tile([128, 2, BC], BF16, tag="u_i")
            # CRF chain state: [parity, chain(0=fwd,1=bwd), BC]
            CH = bp.tile([NB, 2, 2, BC], BF16, tag="CH")
            P2 = bp.tile([NB, BC], F32, tag="P2")
            acc_f = bp.tile([1, BC], F32, tag="acc_f")
            acc_b = bp.tile([1, BC], F32, tag="acc_b")
            rec_f = bp.tile([1, BC], F32, tag="rec_f")
            rec_fb = bp.tile([1, BC], BF16, tag="rec_fb")
            rec_b = bp.tile([1, BC], F32, tag="rec_b")
            rec_bb = bp.tile([1, BC], BF16, tag="rec_bb")
            lg_f = bp.tile([1, BC], F32, tag="lg_f")
            lg_b = bp.tile([1, BC], F32, tag="lg_b")
            res_s = bp.tile([4, BC], F32, tag="res_s")

            # ---- small loads ----
            nc.sync.dma_start(out=idx_f[:, :], in_=idx_d[:])
            nc.sync.dma_start(out=eye_s[:, :], in_=eye_d[:])
            nc.sync.dma_start(out=wih[:, :, :], in_=wih_d[:])
            nc.sync.dma_start(out=whh[:, :, :], in_=whh_d[:])
            nc.sync.dma_start(out=wc[:, :, :], in_=wc_d[:])
            nc.sync.dma_start(out=bc_s[:, :], in_=bc_d[:])
            nc.sync.dma_start(out=esm[:, :], in_=esm_d[:])
            nc.sync.dma_start(out=est[:, :], in_=est_d[:])
            nc.sync.dma_start(out=etb[:, :], in_=etb_d[:])
            nc.sync.dma_start(out=veb[:, :], in_=veb_d[:])
            nc.sync.dma_start(out=ones19[:, :], in_=ones19_d[:])
            nc.sync.dma_start(out=ones19f[:, :], in_=ones19f_d[:])
            nc.sync.dma_start(out=one1x19[:, :], in_=one1x19_d[:])

            nc.vector.memset(acc_f[:, :], 0.0)
            nc.vector.memset(acc_b[:, :], 0.0)

            # =========== phase A: gather + both LSTMs, interleaved ==========
            # one nat tile per chunk: gather DMAs then carry no pool-WAR
            # waits, so no multi-us gpsimd Drains throttle the gather
            with tc.tile_pool(name="gat", bufs=NCHUNK) as gp, \
                 tc.tile_pool(name="gps", bufs=2, space="PSUM") as gpp, \
                 tc.tile_pool(name="psE", bufs=2, space="PSUM") as pe_pool, \
                 tc.tile_pool(name="psA", bufs=1, space="PSUM") as pa:
                GA = pa.tile([128, GCH, TBLK, BC], F32, tag="GA")
                GB = pa.tile([128, GCH, TBLK, BC], F32, tag="GB")
                gbuf = (GA, GB)

                nat_tiles = {}

                def gather_dma(c):
                    nat = gp.tile([128, EPAD], BF16, tag="nat")
                    nc.gpsimd.indirect_dma_start(
                        out=nat[:, :], out_offset=None,
                        in_=emb_d[:, :],
                        in_offset=bass.IndirectOffsetOnAxis(
                            ap=idx_f[:, c:c + 1], axis=0),
                    )
                    nat_tiles[c] = nat

                def gather_tp(c):
                    nat = nat_tiles.pop(c)
                    tp = gpp.tile([128, 128], BF16, tag="tp")
                    nc.tensor.transpose(tp[:, :], nat[:, :], eye_s[:, :])
                    nc.vector.tensor_copy(
                        xeT_f[:, c * 128:(c + 1) * 128], tp[:, :])

                xe3 = xeT_f[0:KP, :].rearrange("p (t b) -> p t b", b=BC)

                def bulk_mm(k, c):
                    G = gbuf[k % 2]
                    if c % 2 == 0:
                        rhs = xe3[:, k * TBLK:(k + 1) * TBLK, :]
                    else:
                        hi = SL - 1 - k * TBLK
                        rhs = (xe3[:, hi:hi - TBLK:-1, :]
                               if hi - TBLK >= 0 else xe3[:, hi::-1, :])
                    nc.tensor.matmul(
                        G[:, c, :, :], wih[0:KP, c, :], rhs,
                        start=True, stop=False, skip_group_check=True,
                    )

                MULT = mybir.AluOpType.mult
                ADD = mybir.AluOpType.add
                SUB = mybir.AluOpType.subtract

                def step(t):
                    G = gbuf[(t // TBLK) % 2]
                    tau = t % TBLK
                    rd = t * BC
                    gs = gates_s0 if t % 2 == 0 else gates_s1
                    th = th0 if t % 2 == 0 else th1
                    if t > 0:
                        # recurrent matmuls: f,i,g first, then o
                        for c in (0, 1, 2, 3, 4, 5):
                            d = c % 2
                            nc.tensor.matmul(
                                G[:, c, tau, :], whh[0:I, c, :],
                                h_all[0:I, d, rd:rd + BC],
                                start=False, stop=True, skip_group_check=True,
                            )
                    # one sigmoid covers f,i,g (g pre-acts carry a 2x host
                    # scale, so sigma here encodes tanh(g) = 2*sigma(2g)-1)
                    nc.scalar.activation(gs[:, 0:6, :], G[:, 0:6, tau, :], SIG)
                    if t > 0:
                        for c in (6, 7):
                            d = c % 2
                            nc.tensor.matmul(
                                G[:, c, tau, :], whh[0:I, c, :],
                                h_all[0:I, d, rd:rd + BC],
                                start=False, stop=True, skip_group_check=True,
                            )
                    # sigmoid(o) off the critical path
                    nc.scalar.activation(gs[:, 6:8, :], G[:, 6:8, tau, :], SIG)
                    # cellc tracks the HALF-cell d = c/2, making the cell
                    # update end in a plain add: d = f*d + (sigma_g-0.5)*i
                    # [= f*c/2 + tanh(g)/2*i]; tanh(c) = tanh(2d) via scale
                    nc.vector.scalar_tensor_tensor(
                        u_i[:, :, :], gs[:, 4:6, :], 0.5, gs[:, 2:4, :],
                        op0=SUB, op1=MULT)
                    if t > 0:
                        nc.vector.tensor_mul(
                            u_f[:, :, :], gs[:, 0:2, :], cellc[:, :, :])
                        nc.vector.tensor_add(
                            cellc[:, :, :], u_f[:, :, :], u_i[:, :, :])
                    else:
                        nc.vector.tensor_copy(cellc[:, :, :], u_i[:, :, :])
                    nc.scalar.activation(th[:, :, :], cellc[:, :, :], TANH,
                                         scale=2.0)
                    wr = (t + 1) * BC
                    nc.vector.tensor_mul(
                        h_all[:, :, wr:wr + BC], gs[:, 6:8, :], th[:, :, :]
                    )

                import os
                _ALLGATHER = bool(int(os.environ.get("KV2_ALLGATHER", "0")))
                if _ALLGATHER:
                    for j in range(NCHUNK):
                        gather_dma(j)
                        gather_tp(j)
                else:
                    # prologue: DMA chunks for blocks 0..5 (both ends,
                    # interleaved so early transposes unblock first),
                    # transpose chunks for blocks 0..3
                    for j in (0, 1, 2, 3, 4, 5):
                        gather_dma(j)
                        gather_dma(NCHUNK - 1 - j)
                    for j in (0, 1, 2, 3):
                        gather_tp(j)
                        gather_tp(NCHUNK - 1 - j)
                for c in range(GCH):
                    bulk_mm(0, c)
                for c in range(GCH):
                    bulk_mm(1, c)

                # in-A emission production: emission matmuls for the middle
                # blocks run here (h for block j is complete from iter
                # max(16j+15, 496-16j); PE has slack), storing pre-exp
                # emissions to SBUF in bf16. Phase C then only runs the EXPs
                # (no activation-table thrash: exp stays out of phase A).
                EBLK = 16
                HBq = 4  # tokens per emission quarter-matmul
                hb_ap_a = h_all[0:I, 1, :].rearrange("p (t b) -> p t b", b=BC)
                ema_tiles = {}

                def ema_quarter(blk, part):
                    t0 = blk * EBLK
                    if blk not in ema_tiles:
                        ema_new = pe_pool.tile([NB, EBLK * BC], F32,
                                               tag="ema_ps")
                        ema_tiles[blk] = ema_new
                    em_ps = ema_tiles[blk]
                    th0_ = t0 + part * HBq
                    sl = slice(part * HBq * BC, (part + 1) * HBq * BC)
                    # hf for token t lives at col (t+1)*BC
                    nc.tensor.matmul(
                        em_ps[:, sl], wc[0:I, 0, :],
                        h_all[0:I, 0, (th0_ + 1) * BC:(th0_ + 1 + HBq) * BC],
                        start=True, stop=False, skip_group_check=True,
                    )
                    # hb for token t lives at round (SL-1-t): col (SL-t)*BC
                    nc.tensor.matmul(
                        em_ps[:, sl].rearrange("p (t b) -> p t b", b=BC),
                        wc[0:I, 1, :],
                        hb_ap_a[:, SL - th0_:SL - th0_ - HBq:-1, :],
                        start=False, stop=True, skip_group_check=True,
                    )

                def ema_cast(blk):
                    t0 = blk * EBLK
                    em_ps = ema_tiles.pop(blk)
                    nc.vector.tensor_copy(
                        emstore[:, t0 * BC:(t0 + EBLK) * BC], em_ps[:, :])

                def ema_slot(t):
                    # window k covers iters [264+16k, 280+16k) and produces
                    # blocks lo=15-k and hi=16+k. Per-part readiness (h_f[t]
                    # at iter t, h_b[t] at iter 511-t): lo part p ready at
                    # iter 271+16k-4p (reverse order), hi part p at 259+16k+4p
                    if not (264 <= t < 504):
                        return
                    kwin, loc = divmod(t - 264, 16)
                    lo, hi = 15 - kwin, 16 + kwin
                    if loc == 0:
                        ema_quarter(lo, 3)
                    elif loc == 1:
                        ema_quarter(lo, 2)
                    elif loc == 2:
                        ema_quarter(hi, 0)
                    elif loc == 3:
                        ema_quarter(lo, 1)
                    elif loc == 4:
                        ema_quarter(hi, 1)
                    elif loc == 6:
                        ema_quarter(hi, 2)
                    elif loc == 8:
                        ema_quarter(lo, 0)
                    elif loc == 9:
                        ema_quarter(hi, 3)
                    elif loc == 10:
                        ema_cast(lo)
                    elif loc == 12:
                        ema_cast(hi)

                # chunk c serves fwd block c and bwd block NCHUNK-1-c, so every
                # chunk must be resident before block NBLK//2. DMA the chunk
                # used by block j at block j-6, transpose it at block j-4 —
                # ~2 blocks of runway so transposes never stall PE on gpsimd.
                mid = NCHUNK // 2 - 1  # 63
                for k in range(NBLK):
                    for tau in range(TBLK):
                        step(k * TBLK + tau)
                        ema_slot(k * TBLK + tau)
                        # spread helper work across the 4 steps of the block
                        if tau == 0:
                            if not _ALLGATHER and 6 <= k + 6 <= mid:
                                gather_dma(k + 6)
                            if k + 2 < NBLK:
                                bulk_mm(k + 2, 0)
                                bulk_mm(k + 2, 2)
                        elif tau == 1:
                            if not _ALLGATHER and 4 <= k + 4 <= mid:
                                gather_tp(k + 4)
                            if k + 2 < NBLK:
                                bulk_mm(k + 2, 4)
                                bulk_mm(k + 2, 6)
                        elif tau == 2:
                            cb = NCHUNK - 7 - k
                            if not _ALLGATHER and cb >= mid + 1:
                                gather_dma(cb)
                            if k + 2 < NBLK:
                                bulk_mm(k + 2, 1)
                                bulk_mm(k + 2, 3)
                        else:
                            cb = NCHUNK - 5 - k
                            if not _ALLGATHER and mid + 1 <= cb <= NCHUNK - 5:
                                gather_tp(cb)
                            if k + 2 < NBLK:
                                bulk_mm(k + 2, 5)
                                bulk_mm(k + 2, 7)

            # ==== phase B+C: Y = exp(em + bc) from the in-A emission store,
            # ==== interleaved with the CRF partition chains (exp and ln
            # ==== share the natural_log_exp activation table). Only blocks
            # ==== 0 and 31 (which need the very last LSTM iters) run their
            # ==== matmuls here.
            with tc.tile_pool(name="psB", bufs=2, space="PSUM") as pb, \
                 tc.tile_pool(name="psC", bufs=2, space="PSUM") as pc, \
                 tc.tile_pool(name="psC2", bufs=1, space="PSUM") as pc2:

                def emit_block(blk):
                    t0 = blk * EBLK
                    em_new = pb.tile([NB, EBLK * BC], F32, tag="em_ps")
                    for part in range(4):
                        th0_ = t0 + part * HBq
                        sl = slice(part * HBq * BC, (part + 1) * HBq * BC)
                        nc.tensor.matmul(
                            em_new[:, sl], wc[0:I, 0, :],
                            h_all[0:I, 0,
                                  (th0_ + 1) * BC:(th0_ + 1 + HBq) * BC],
                            start=True, stop=False, skip_group_check=True,
                        )
                        nc.tensor.matmul(
                            em_new[:, sl].rearrange("p (t b) -> p t b", b=BC),
                            wc[0:I, 1, :],
                            hb_ap_a[:, SL - th0_:SL - th0_ - HBq:-1, :],
                            start=False, stop=True, skip_group_check=True,
                        )
                    nc.scalar.activation(
                        Y[:, t0 * BC:(t0 + EBLK) * BC], em_new[:, :], EXP,
                        bias=bc_s[:, 0:1]
                    )

                def emit_exp(blk):
                    # exp over the bf16 pre-emissions stored during phase A
                    t0 = blk * EBLK
                    nc.scalar.activation(
                        Y[:, t0 * BC:(t0 + EBLK) * BC],
                        emstore[:, t0 * BC:(t0 + EBLK) * BC], EXP,
                        bias=bc_s[:, 0:1]
                    )

                emit_block(0)
                emit_block(31)
                # W0 = Y_0 * exp(T[BOS,:]) ; V = veb * Y_last
                # chain state in CH[parity, chain, :]: hop r reads parity
                # (r-1)%2, writes r%2 — no in-place WAR; the fwd and bwd hop
                # multiplies merge into ONE strided-AP tensor_tensor per r
                Yp = Y[0:NB, :].rearrange("p (t b) -> p t b", b=BC)
                nc.vector.tensor_scalar_mul(CH[:, 0, 0, :], Y[0:NB, 0:BC],
                                            etb[:, 0:1])
                nc.vector.tensor_mul(CH[:, 1, 1, :], veb[:, :],
                                     Y[0:NB, (SL - 1) * BC:SL * BC])

                # small phase-C PSUM tensors: one bank per chain so the fwd
                # and bwd renorm pipelines don't false-serialize on a bank
                crfF = pc2.tile([NB, 3 * BC], F32, tag="crfF")
                crfB = pc2.tile([NB, 2 * BC], F32, tag="crfB")
                rf_ps = crfF[:, 0:BC]
                sf_ps = crfF[0:1, BC:2 * BC]
                dot_ps = crfF[0:1, 2 * BC:3 * BC]
                rb_ps = crfB[:, 0:BC]
                sb_ps = crfB[0:1, BC:2 * BC]

                def renorm_snap(w_sb, s_ps):
                    # s = ones19^T @ w (PE, off the recurrence chain)
                    nc.tensor.matmul(s_ps, ones19[:, :], w_sb[:, :],
                                     skip_group_check=True)

                def renorm_mid(s_ps, rec, recb, r_ps, lg, acc):
                    nc.vector.reciprocal(rec[:, :], s_ps)
                    nc.vector.tensor_copy(recb[:, :], rec[:, :])
                    nc.tensor.matmul(r_ps, one1x19[:, :], recb[:, :],
                                     skip_group_check=True)
                    nc.scalar.activation(lg[:, :], s_ps, LOG)
                    nc.vector.tensor_add(acc[:, :], acc[:, :], lg[:, :])

                wb_prev = None
                HALF = SL // 2
                for r in range(HALF):
                    p, q = r % 2, (r + 1) % 2  # dst / src parity
                    ty = SL - 2 - r  # next Y column for backward chain
                    # just-in-time Y production (exp only): low block kk+1 at
                    # r=16kk+6, high block 30-kk at r=16kk+11 (blocks 0, 31
                    # are fully produced before the loop)
                    kk, jj = divmod(r, RENORM)
                    if kk <= 14 and jj == 6:
                        emit_exp(kk + 1)
                    elif kk <= 14 and jj == 11:
                        emit_exp(30 - kk)
                    wfb_ps = pc.tile([NB, 2, BC], F32, tag="wfb_ps")
                    # backward chain mm (always)
                    nc.tensor.matmul(wfb_ps[:, 1, :], est[:, :],
                                     CH[:, q, 1, :], skip_group_check=True)
                    # forward chain mm: t = r = 1..HALF-1
                    if r >= 1:
                        nc.tensor.matmul(wfb_ps[:, 0, :], esm[:, :],
                                         CH[:, q, 0, :], skip_group_check=True)
                    if r % RENORM == 0 and r >= RENORM:
                        renorm_snap(CH[:, q, 0, :], sf_ps)
                        renorm_snap(CH[:, q, 1, :], sb_ps)
                    # hop multiplies: one strided TT covers both chains
                    if 1 <= r < HALF - 1:
                        nc.vector.tensor_mul(
                            CH[:, p, :, :], wfb_ps[:, :, :],
                            Yp[:, r:ty + 1:(ty - r), :])
                    elif r == 0:
                        nc.vector.tensor_mul(
                            CH[:, p, 1, :], wfb_ps[:, 1, :],
                            Y[0:NB, ty * BC:(ty + 1) * BC])
                    else:  # r == HALF-1: forward hop only
                        nc.vector.tensor_mul(
                            CH[:, p, 0, :], wfb_ps[:, 0, :],
                            Y[0:NB, r * BC:(r + 1) * BC])
                    if r % RENORM == 0 and r >= RENORM:
                        renorm_mid(sf_ps, rec_f, rec_fb, rf_ps, lg_f, acc_f)
                        renorm_mid(sb_ps, rec_b, rec_bb, rb_ps, lg_b, acc_b)
                    if r >= RENORM + 3 and (r - 3) % RENORM == 0:
                        nc.vector.tensor_mul(CH[:, p, 0, :], CH[:, p, 0, :],
                                             rf_ps)
                        nc.vector.tensor_mul(CH[:, p, 1, :], CH[:, p, 1, :],
                                             rb_ps)
                    wb_prev = wfb_ps

                # meet at t=HALF-1: P2 = Wf_{HALF-1} * beta_{HALF-1}
                nc.vector.tensor_mul(P2[:, :], CH[:, (HALF - 1) % 2, 0, :],
                                     wb_prev[:, 1, :])
                nc.tensor.matmul(dot_ps, ones19f[:, :], P2[:, :],
                                 skip_group_check=True)
                nc.scalar.activation(res_s[0:1, :], dot_ps, LOG)

            nc.sync.dma_start(out=y_out[:], in_=Y[:, :])
            nc.sync.dma_start(out=res_out[0:1], in_=res_s[0:1, :])
            nc.sync.dma_start(out=res_out[1:2], in_=acc_f[:, :])
            nc.sync.dma_start(out=res_out[2:3], in_=acc_b[:, :])

    return nc


def _split_waits(nc):
    """Walrus codegen allows ~1 sync-wait on compute instrs; move excess
    waits onto injected same-engine Drain instructions (which allow many).

    Keep the wait most likely to be satisfied LAST inline on the compute
    instruction (a cross-engine producer), and drain the early-satisfied
    ones (same-engine program-order waits) first — a drain blocked on the
    critical producer adds ~70-90ns of serial queue decode vs an inline
    wait that fires as soon as the semaphore lands."""
    from concourse import mybir as mb

    def sem_engine(w):
        nm = getattr(w, 'ant_name', '') or ''
        return nm.split('_')[0]

    eng_name = {
        mb.EngineType.PE: 'PE', mb.EngineType.Activation: 'Activation',
        mb.EngineType.DVE: 'DVE', mb.EngineType.Pool: 'Pool',
        mb.EngineType.SP: 'SP',
    }
    n = 0
    for f in nc.m.functions:
        for blk in f.blocks:
            insts = blk.instructions
            new_list = []
            for ins in insts:
                si = ins.sync_info
                if si is not None and si.on_wait and len(si.on_wait) > 1:
                    waits = list(si.on_wait)
                    own = eng_name.get(ins.engine, '?')
                    cross = [w for w in waits if sem_engine(w) != own]
                    selfw = [w for w in waits if sem_engine(w) == own]
                    inline = [cross[-1]] if cross else [waits[-1]]
                    rest = [w for w in waits if w is not inline[0]]
                    # self-engine waits first (satisfied early), cross after
                    rest.sort(key=lambda w: 0 if sem_engine(w) == own else 1)
                    for w in rest:
                        d = mb.InstDrain(
                            name=f"{ins.name}-ws{n}", ins=[], outs=[])
                        d.engine = ins.engine
                        d.sync_info = mb.SyncInfo(on_wait=[w], on_update=[])
                        new_list.append(d)
                        n += 1
                    ins.sync_info = mb.SyncInfo(
                        on_wait=inline, on_update=list(si.on_update))
                new_list.append(ins)
            del insts[:]
            insts.extend(new_list)
    return n


def _prep_host(inputs):
    emb = np.asarray(inputs["emb"], np.float32)
    T = np.asarray(inputs["transitions"], np.float32)
    W1 = np.asarray(inputs["W1"], np.float32)
    b1 = np.asarray(inputs["b1"], np.float32)
    W2 = np.asarray(inputs["W2"], np.float32)
    b2 = np.asarray(inputs["b2"], np.float32)

    emb_pad = np.zeros((V, EPAD), np.float32)
    emb_pad[:, 0:I] = emb
    emb_pad[:, I] = 1.0  # bias-aug ones row

    # gate reorder: pytorch [i,f,g,o] -> ours [f,i,g,o]
    perm = np.concatenate([np.arange(I, 2 * I), np.arange(0, I),
                           np.arange(2 * I, 3 * I), np.arange(3 * I, 4 * I)])

    def pack_dir(Wih, Whh, bih, bhh):
        Wih, Whh = Wih[perm].copy(), Whh[perm].copy()
        bias = (bih + bhh)[perm].copy()
        # 2x the g-gate pre-acts: kernel computes tanh(g) as 2*sigma(2g)-1
        Wih[2 * I:3 * I] *= 2.0
        Whh[2 * I:3 * I] *= 2.0
        bias[2 * I:3 * I] *= 2.0
        wih = np.zeros((4, 128, 128), np.float32)  # [gamma, k, m]
        whh = np.zeros((4, 128, 128), np.float32)
        for g in range(4):
            wih[g, 0:I, 0:I] = Wih[g * I:(g + 1) * I].T
            wih[g, I, 0:I] = bias[g * I:(g + 1) * I]
            whh[g, 0:I, 0:I] = Whh[g * I:(g + 1) * I].T
        return wih, whh

    wih_f, whh_f = pack_dir(np.asarray(inputs["Wih_f"], np.float32),
                            np.asarray(inputs["Whh_f"], np.float32),
                            np.asarray(inputs["bih_f"], np.float32),
                            np.asarray(inputs["bhh_f"], np.float32))
    wih_b, whh_b = pack_dir(np.asarray(inputs["Wih_b"], np.float32),
                            np.asarray(inputs["Whh_b"], np.float32),
                            np.asarray(inputs["bih_b"], np.float32),
                            np.asarray(inputs["bhh_b"], np.float32))

    wih = np.zeros((128, GCH, 128), np.float32)
    whh = np.zeros((128, GCH, 128), np.float32)
    for g in range(4):
        wih[:, g * 2 + 0, :] = wih_f[g]
        wih[:, g * 2 + 1, :] = wih_b[g]
        whh[:, g * 2 + 0, :] = whh_f[g]
        whh[:, g * 2 + 1, :] = whh_b[g]

    Wc = W2 @ W1                      # [19, 200]
    bcv = W2 @ b1 + b2                # [19]
    wc = np.zeros((128, 2, NB), np.float32)
    wc[0:I, 0, :] = Wc[:, 0:I].T
    wc[0:I, 1, :] = Wc[:, I:2 * I].T

    c0 = float(np.log(np.sum(np.exp(bcv))))
    esm = np.exp(T - c0)
    est = esm.T.copy()
    etb = np.exp(T[BOS, :]).reshape(NB, 1)
    veb = np.broadcast_to(np.exp(T[:, EOS]).reshape(NB, 1), (NB, BC)).copy()

    bf = ml_dtypes.bfloat16
    common = {
        "emb_pad": emb_pad.astype(bf),
        "wih": wih.astype(bf),
        "whh": whh.astype(bf),
        "wc": wc.astype(bf),
        "bc": bcv.reshape(NB, 1).astype(np.float32),
        "esm": esm.astype(bf),
        "est": est.astype(bf),
        "etb": etb.astype(np.float32),
        "veb": veb.astype(np.float32),
        "ones19": np.ones((NB, 1), bf),
        "ones19f": np.ones((NB, 1), np.float32),
        "one1x19": np.ones((1, NB), bf),
    }
    return common, c0, bcv


def kernel(**inputs):
    x = np.asarray(inputs["x"]).reshape(B, S).astype(np.int64)
    target = np.asarray(inputs["target"]).reshape(B, S).astype(np.int64)
    T = np.asarray(inputs["transitions"], np.float32)

    common, c0, bcv = _prep_host(inputs)

    common["eye"] = np.eye(128, dtype=ml_dtypes.bfloat16)
    in_maps = []
    for c in range(NCORES):
        xs = x[c * BC:(c + 1) * BC]  # [BC, S]
        # fwd token order: col t*BC + b  -> x[b, t]
        idx_fwd = xs.T.reshape(-1).astype(np.int32)
        idxs = idx_fwd.reshape(NT // 128, 128).T.copy()
        in_maps.append({**common, "idxs": idxs})

    if "nc" not in _CACHE:
        nc0 = _build_nc()
        _split_waits(nc0)
        mybir.codegen_inst_isa_subclasses(nc0)
        _CACHE["nc"] = nc0
    nc = _CACHE["nc"]
    _CACHE["in_maps"] = in_maps

    results = run_bass_kernel_spmd(nc, in_maps, list(range(NCORES))).results

    # host combine
    t_sc = (T[target[:, :-1], target[:, 1:]].sum(1)
            + T[BOS, target[:, 0]] + T[target[:, -1], EOS])  # [B]

    losses = np.zeros(B, np.float64)
    for c in range(NCORES):
        yv = np.asarray(results[c]["y_out"], ml_dtypes.bfloat16).astype(np.float32)
        res = np.asarray(results[c]["res"], np.float32)
        logY = np.log(yv).reshape(NB, S, BC)  # log Y = em + bc - c0... (em+bc)
        tg = target[c * BC:(c + 1) * BC]      # [BC, S]
        bi = np.arange(BC)
        e_sc = np.zeros(BC, np.float64)
        for t in range(S):
            e_sc += logY[tg[:, t], t, bi]
        partition = res[0] + res[1] + res[2] + (S - 1) * c0
        losses[c * BC:(c + 1) * BC] = (
            e_sc + t_sc[c * BC:(c + 1) * BC] - partition
        )
    return np.float32(-losses.mean())



# revision 31
# speedup vs baseline: 1.0027x; 1.0027x over previous
"""BiLSTM+CRF NLL loss kernel for 8 Trainium2 NeuronCores (v3).

Sharding: data-parallel on batch (32 sequences per core). Each core runs the
full BiLSTM + emission + CRF forward/backward partition recurrences for its
shard; host combines per-core partials into the scalar loss.

v3 vs v2 (trace: 2551ns/step chain = mm,σ,tanh_g,mul,add,tanh_c,mul_h):
- tanh(g) removed from the serial ACT chain: g pre-acts are scaled 2x on the
  host so one sigmoid instruction covers f,i,g (tanh(g) = 2σ(2g)-1), and the
  cell update becomes c = f*c + 2(σ_g-0.5)*i via fused scalar_tensor_tensor
- cell/gate DVE pipeline in bf16 (2x DVE rate), cell updated in place in a
  fixed tile (no cross-engine WAR)
- CRF fwd/bwd hop multiplies merged into one strided-AP tensor_tensor
"""

import numpy as np
import ml_dtypes

import concourse.bass as bass
import concourse.tile as tile
from concourse import mybir
from concourse.bass_utils import run_bass_kernel_spmd

F32 = mybir.dt.float32
BF16 = mybir.dt.bfloat16

B, S, V, I, NB = 256, 512, 30000, 100, 19
BOS, EOS = 17, 18
NCORES = 8
BC = B // NCORES          # 32 sequences per core
NT = BC * S               # 16384 tokens per core
KP = I + 1                # 101: embedding dims + ones row (bias aug)
EPAD = 128                # padded embedding row length
RENORM = 16               # CRF renorm interval
TBLK = 4                  # steps per PSUM gate block
GCH = 8                   # gate chunks: (gamma in [g,f,i,o]) x (dir in [f,b])

_CACHE = {}


def _build_nc(s_len=S):
    SL = s_len
    NTL = BC * SL
    NBLK = SL // TBLK
    NCHUNK = NTL // 128

    nc = bass.Bass()

    # ---- dram I/O ----
    emb_d = nc.dram_tensor("emb_pad", [V, EPAD], BF16, kind="ExternalInput")
    idx_d = nc.dram_tensor("idxs", [128, NTL // 128], mybir.dt.int32, kind="ExternalInput")
    eye_d = nc.dram_tensor("eye", [128, 128], BF16, kind="ExternalInput")
    wih_d = nc.dram_tensor("wih", [128, GCH, 128], BF16, kind="ExternalInput")
    whh_d = nc.dram_tensor("whh", [128, GCH, 128], BF16, kind="ExternalInput")
    wc_d = nc.dram_tensor("wc", [128, 2, NB], BF16, kind="ExternalInput")
    bc_d = nc.dram_tensor("bc", [NB, 1], F32, kind="ExternalInput")
    esm_d = nc.dram_tensor("esm", [NB, NB], BF16, kind="ExternalInput")
    est_d = nc.dram_tensor("est", [NB, NB], BF16, kind="ExternalInput")
    etb_d = nc.dram_tensor("etb", [NB, 1], F32, kind="ExternalInput")
    veb_d = nc.dram_tensor("veb", [NB, BC], F32, kind="ExternalInput")
    ones19_d = nc.dram_tensor("ones19", [NB, 1], BF16, kind="ExternalInput")
    ones19f_d = nc.dram_tensor("ones19f", [NB, 1], F32, kind="ExternalInput")
    one1x19_d = nc.dram_tensor("one1x19", [1, NB], BF16, kind="ExternalInput")

    y_out = nc.dram_tensor("y_out", [NB, NTL], BF16, kind="ExternalOutput")
    res_out = nc.dram_tensor("res", [4, BC], F32, kind="ExternalOutput")

    SIG = mybir.ActivationFunctionType.Sigmoid
    TANH = mybir.ActivationFunctionType.Tanh
    EXP = mybir.ActivationFunctionType.Exp
    LOG = mybir.ActivationFunctionType.Ln

    with tile.TileContext(nc) as tc:
        with tc.tile_pool(name="big", bufs=1) as bp:
            xeT_f = bp.tile([128, NTL], BF16, tag="xeT_f")
            emstore = bp.tile([NB, NTL], BF16, tag="emstore")
            eye_s = bp.tile([128, 128], BF16, tag="eye_s")
            # h storage: col (t+1)*32 = h after step t; col 0 = h(-1)=0
            h_all = bp.tile([128, 2, NTL + BC], BF16, tag="h_all")
            Y = bp.tile([NB, NTL], BF16, tag="Y")
            idx_f = bp.tile([128, NTL // 128], mybir.dt.int32, tag="idx_f")
            wih = bp.tile([128, GCH, 128], BF16, tag="wih")
            whh = bp.tile([128, GCH, 128], BF16, tag="whh")
            wc = bp.tile([128, 2, NB], BF16, tag="wc")
            bc_s = bp.tile([NB, 1], F32, tag="bc_s")
            esm = bp.tile([NB, NB], BF16, tag="esm")
            est = bp.tile([NB, NB], BF16, tag="est")
            etb = bp.tile([NB, 1], F32, tag="etb")
            veb = bp.tile([NB, BC], F32, tag="veb")
            ones19 = bp.tile([NB, 1], BF16, tag="ones19")
            ones19f = bp.tile([NB, 1], F32, tag="ones19f")
            one1x19 = bp.tile([1, NB], BF16, tag="one1x19")
            gates_s0 = bp.tile([128, GCH, BC], BF16, tag="gates_s0")
            gates_s1 = bp.tile([128, GCH, BC], BF16, tag="gates_s1")
            cellc = bp.tile([128, 2, BC], BF16, tag="cellc")  # c, in-place
            th0 = bp.tile([128, 2, BC], BF16, tag="th0")
            th1 = bp.tile([128, 2, BC], BF16, tag="th1")
            u_f = bp.tile([128, 2, BC], BF16, tag="u_f")
            u_i = bp.tile([128, 2, BC], BF16, tag="u_i")
            # CRF chain state: [parity, chain(0=fwd,1=bwd), BC]
            CH = bp.tile([NB, 2, 2, BC], BF16, tag="CH")
            P2 = bp.tile([NB, BC], F32, tag="P2")
            acc_f = bp.tile([1, BC], F32, tag="acc_f")
            acc_b = bp.tile([1, BC], F32, tag="acc_b")
            rec_f = bp.tile([1, BC], F32, tag="rec_f")
            rec_fb = bp.tile([1, BC], BF16, tag="rec_fb")
            rec_b = bp.tile([1, BC], F32, tag="rec_b")
            rec_bb = bp.tile([1, BC], BF16, tag="rec_bb")
            lg_f = bp.tile([1, BC], F32, tag="lg_f")
            lg_b = bp.tile([1, BC], F32, tag="lg_b")
            res_s = bp.tile([4, BC], F32, tag="res_s")

            # ---- small loads ----
            nc.sync.dma_start(out=idx_f[:, :], in_=idx_d[:])
            nc.sync.dma_start(out=eye_s[:, :], in_=eye_d[:])
            nc.sync.dma_start(out=wih[:, :, :], in_=wih_d[:])
            nc.sync.dma_start(out=whh[:, :, :], in_=whh_d[:])
            nc.sync.dma_start(out=wc[:, :, :], in_=wc_d[:])
            nc.sync.dma_start(out=bc_s[:, :], in_=bc_d[:])
            nc.sync.dma_start(out=esm[:, :], in_=esm_d[:])
            nc.sync.dma_start(out=est[:, :], in_=est_d[:])
            nc.sync.dma_start(out=etb[:, :], in_=etb_d[:])
            nc.sync.dma_start(out=veb[:, :], in_=veb_d[:])
            nc.sync.dma_start(out=ones19[:, :], in_=ones19_d[:])
            nc.sync.dma_start(out=ones19f[:, :], in_=ones19f_d[:])
            nc.sync.dma_start(out=one1x19[:, :], in_=one1x19_d[:])

            nc.vector.memset(acc_f[:, :], 0.0)
            nc.vector.memset(acc_b[:, :], 0.0)

            # =========== phase A: gather + both LSTMs, interleaved ==========
            # one nat tile per chunk: gather DMAs then carry no pool-WAR
            # waits, so no multi-us gpsimd Drains throttle the gather
            with tc.tile_pool(name="gat", bufs=NCHUNK) as gp, \
                 tc.tile_pool(name="gps", bufs=2, space="PSUM") as gpp, \
                 tc.tile_pool(name="psE", bufs=2, space="PSUM") as pe_pool, \
                 tc.tile_pool(name="psA", bufs=1, space="PSUM") as pa:
                GA = pa.tile([128, GCH, TBLK, BC], F32, tag="GA")
                GB = pa.tile([128, GCH, TBLK, BC], F32, tag="GB")
                gbuf = (GA, GB)

                nat_tiles = {}

                def gather_dma(c):
                    nat = gp.tile([128, EPAD], BF16, tag="nat")
                    nc.gpsimd.indirect_dma_start(
                        out=nat[:, :], out_offset=None,
                        in_=emb_d[:, :],
                        in_offset=bass.IndirectOffsetOnAxis(
                            ap=idx_f[:, c:c + 1], axis=0),
                    )
                    nat_tiles[c] = nat

                def gather_tp(c):
                    nat = nat_tiles.pop(c)
                    tp = gpp.tile([128, 128], BF16, tag="tp")
                    nc.tensor.transpose(tp[:, :], nat[:, :], eye_s[:, :])
                    nc.vector.tensor_copy(
                        xeT_f[:, c * 128:(c + 1) * 128], tp[:, :])

                xe3 = xeT_f[0:KP, :].rearrange("p (t b) -> p t b", b=BC)

                def bulk_mm(k, c):
                    G = gbuf[k % 2]
                    if c % 2 == 0:
                        rhs = xe3[:, k * TBLK:(k + 1) * TBLK, :]
                    else:
                        hi = SL - 1 - k * TBLK
                        rhs = (xe3[:, hi:hi - TBLK:-1, :]
                               if hi - TBLK >= 0 else xe3[:, hi::-1, :])
                    nc.tensor.matmul(
                        G[:, c, :, :], wih[0:KP, c, :], rhs,
                        start=True, stop=False, skip_group_check=True,
                    )

                MULT = mybir.AluOpType.mult
                ADD = mybir.AluOpType.add
                SUB = mybir.AluOpType.subtract

                def step(t):
                    G = gbuf[(t // TBLK) % 2]
                    tau = t % TBLK
                    rd = t * BC
                    gs = gates_s0 if t % 2 == 0 else gates_s1
                    th = th0 if t % 2 == 0 else th1
                    if t > 0:
                        # recurrent matmuls: f,i,g first, then o
                        for c in (0, 1, 2, 3, 4, 5):
                            d = c % 2
                            nc.tensor.matmul(
                                G[:, c, tau, :], whh[0:I, c, :],
                                h_all[0:I, d, rd:rd + BC],
                                start=False, stop=True, skip_group_check=True,
                            )
                    # one sigmoid covers f,i,g (g pre-acts carry a 2x host
                    # scale, so sigma here encodes tanh(g) = 2*sigma(2g)-1)
                    nc.scalar.activation(gs[:, 0:6, :], G[:, 0:6, tau, :], SIG)
                    if t > 0:
                        for c in (6, 7):
                            d = c % 2
                            nc.tensor.matmul(
                                G[:, c, tau, :], whh[0:I, c, :],
                                h_all[0:I, d, rd:rd + BC],
                                start=False, stop=True, skip_group_check=True,
                            )
                    # sigmoid(o) off the critical path
                    nc.scalar.activation(gs[:, 6:8, :], G[:, 6:8, tau, :], SIG)
                    # cellc tracks the HALF-cell d = c/2, making the cell
                    # update end in a plain add: d = f*d + (sigma_g-0.5)*i
                    # [= f*c/2 + tanh(g)/2*i]; tanh(c) = tanh(2d) via scale
                    nc.vector.scalar_tensor_tensor(
                        u_i[:, :, :], gs[:, 4:6, :], 0.5, gs[:, 2:4, :],
                        op0=SUB, op1=MULT)
                    if t > 0:
                        nc.vector.tensor_mul(
                            u_f[:, :, :], gs[:, 0:2, :], cellc[:, :, :])
                        nc.vector.tensor_add(
                            cellc[:, :, :], u_f[:, :, :], u_i[:, :, :])
                    else:
                        nc.vector.tensor_copy(cellc[:, :, :], u_i[:, :, :])
                    nc.scalar.activation(th[:, :, :], cellc[:, :, :], TANH,
                                         scale=2.0)
                    wr = (t + 1) * BC
                    nc.vector.tensor_mul(
                        h_all[:, :, wr:wr + BC], gs[:, 6:8, :], th[:, :, :]
                    )

                import os
                _ALLGATHER = bool(int(os.environ.get("KV2_ALLGATHER", "0")))
                if _ALLGATHER:
                    for j in range(NCHUNK):
                        gather_dma(j)
                        gather_tp(j)
                else:
                    # prologue: DMA chunks for blocks 0..9 (both ends,
                    # interleaved so early transposes unblock first),
                    # transpose chunks for blocks 0..7. The deep prefetch
                    # builds gather runway while the pre-transposed blocks
                    # run, so in-loop transposes/copies never stall their
                    # engine queues waiting on an in-flight gather.
                    for j in range(10):
                        gather_dma(j)
                        gather_dma(NCHUNK - 1 - j)
                    for j in range(8):
                        gather_tp(j)
                        gather_tp(NCHUNK - 1 - j)
                for c in range(GCH):
                    bulk_mm(0, c)
                for c in range(GCH):
                    bulk_mm(1, c)

                # in-A emission production: emission matmuls for the middle
                # blocks run here (h for block j is complete from iter
                # max(16j+15, 496-16j); PE has slack), storing pre-exp
                # emissions to SBUF in bf16. Phase C then only runs the EXPs
                # (no activation-table thrash: exp stays out of phase A).
                EBLK = 16
                HBq = 4  # tokens per emission quarter-matmul
                hb_ap_a = h_all[0:I, 1, :].rearrange("p (t b) -> p t b", b=BC)
                ema_tiles = {}

                def ema_quarter(blk, part):
                    t0 = blk * EBLK
                    if blk not in ema_tiles:
                        ema_new = pe_pool.tile([NB, EBLK * BC], F32,
                                               tag="ema_ps")
                        ema_tiles[blk] = ema_new
                    em_ps = ema_tiles[blk]
                    th0_ = t0 + part * HBq
                    sl = slice(part * HBq * BC, (part + 1) * HBq * BC)
                    # hf for token t lives at col (t+1)*BC
                    nc.tensor.matmul(
                        em_ps[:, sl], wc[0:I, 0, :],
                        h_all[0:I, 0, (th0_ + 1) * BC:(th0_ + 1 + HBq) * BC],
                        start=True, stop=False, skip_group_check=True,
                    )
                    # hb for token t lives at round (SL-1-t): col (SL-t)*BC
                    nc.tensor.matmul(
                        em_ps[:, sl].rearrange("p (t b) -> p t b", b=BC),
                        wc[0:I, 1, :],
                        hb_ap_a[:, SL - th0_:SL - th0_ - HBq:-1, :],
                        start=False, stop=True, skip_group_check=True,
                    )

                def ema_cast(blk, half):
                    # halves bound the DVE head-of-line delay to ~370ns
                    t0 = blk * EBLK
                    em_ps = ema_tiles[blk]
                    HC = EBLK * BC // 2
                    nc.vector.tensor_copy(
                        emstore[:, t0 * BC + half * HC:t0 * BC + (half + 1) * HC],
                        em_ps[:, half * HC:(half + 1) * HC])
                    if half == 1:
                        ema_tiles.pop(blk)

                def ema_slot(t):
                    # window k covers iters [264+16k, 280+16k) and produces
                    # blocks lo=15-k and hi=16+k. Per-part readiness (h_f[t]
                    # at iter t, h_b[t] at iter 511-t): lo part p ready at
                    # iter 271+16k-4p (reverse order), hi part p at 259+16k+4p
                    if not (264 <= t < 504):
                        return
                    kwin, loc = divmod(t - 264, 16)
                    lo, hi = 15 - kwin, 16 + kwin
                    if loc == 0:
                        ema_quarter(lo, 3)
                    elif loc == 1:
                        ema_quarter(lo, 2)
                    elif loc == 2:
                        ema_quarter(hi, 0)
                    elif loc == 5:
                        ema_quarter(lo, 1)
                    elif loc == 4:
                        ema_quarter(hi, 1)
                    elif loc == 6:
                        ema_quarter(hi, 2)
                    elif loc == 8:
                        ema_quarter(lo, 0)
                    elif loc == 9:
                        ema_quarter(hi, 3)
                    elif loc == 10:
                        ema_cast(lo, 0)
                    elif loc == 11:
                        ema_cast(lo, 1)
                    elif loc == 12:
                        ema_cast(hi, 0)
                    elif loc == 13:
                        ema_cast(hi, 1)

                # chunk c serves fwd block c and bwd block NCHUNK-1-c, so every
                # chunk must be resident before block NBLK//2. DMA the chunk
                # used by block j at block j-6, transpose it at block j-4 —
                # ~2 blocks of runway so transposes never stall PE on gpsimd.
                mid = NCHUNK // 2 - 1  # 63
                for k in range(NBLK):
                    for tau in range(TBLK):
                        step(k * TBLK + tau)
                        ema_slot(k * TBLK + tau)
                        # spread helper work across the 4 steps of the block
                        if tau == 0:
                            if not _ALLGATHER and 10 <= k + 10 <= mid:
                                gather_dma(k + 10)
                            if k + 2 < NBLK:
                                bulk_mm(k + 2, 0)
                                bulk_mm(k + 2, 2)
                        elif tau == 1:
                            if not _ALLGATHER and 8 <= k + 8 <= mid:
                                gather_tp(k + 8)
                            if k + 2 < NBLK:
                                bulk_mm(k + 2, 4)
                                bulk_mm(k + 2, 6)
                        elif tau == 2:
                            cb = NCHUNK - 11 - k
                            if not _ALLGATHER and cb >= mid + 1:
                                gather_dma(cb)
                            if k + 2 < NBLK:
                                bulk_mm(k + 2, 1)
                                bulk_mm(k + 2, 3)
                        else:
                            cb = NCHUNK - 9 - k
                            if not _ALLGATHER and mid + 1 <= cb <= NCHUNK - 9:
                                gather_tp(cb)
                            if k + 2 < NBLK:
                                bulk_mm(k + 2, 5)
                                bulk_mm(k + 2, 7)

            # ==== phase B+C: Y = exp(em + bc) from the in-A emission store,
            # ==== interleaved with the CRF partition chains (exp and ln
            # ==== share the natural_log_exp activation table). Only blocks
            # ==== 0 and 31 (which need the very last LSTM iters) run their
            # ==== matmuls here.
            with tc.tile_pool(name="psB", bufs=2, space="PSUM") as pb, \
                 tc.tile_pool(name="psC", bufs=2, space="PSUM") as pc, \
                 tc.tile_pool(name="psC2", bufs=1, space="PSUM") as pc2:

                def emit_block(blk):
                    t0 = blk * EBLK
                    em_new = pb.tile([NB, EBLK * BC], F32, tag="em_ps")
                    for part in range(4):
                        th0_ = t0 + part * HBq
                        sl = slice(part * HBq * BC, (part + 1) * HBq * BC)
                        nc.tensor.matmul(
                            em_new[:, sl], wc[0:I, 0, :],
                            h_all[0:I, 0,
                                  (th0_ + 1) * BC:(th0_ + 1 + HBq) * BC],
                            start=True, stop=False, skip_group_check=True,
                        )
                        nc.tensor.matmul(
                            em_new[:, sl].rearrange("p (t b) -> p t b", b=BC),
                            wc[0:I, 1, :],
                            hb_ap_a[:, SL - th0_:SL - th0_ - HBq:-1, :],
                            start=False, stop=True, skip_group_check=True,
                        )
                    nc.scalar.activation(
                        Y[:, t0 * BC:(t0 + EBLK) * BC], em_new[:, :], EXP,
                        bias=bc_s[:, 0:1]
                    )

                def emit_exp(blk):
                    # exp over the bf16 pre-emissions stored during phase A
                    t0 = blk * EBLK
                    nc.scalar.activation(
                        Y[:, t0 * BC:(t0 + EBLK) * BC],
                        emstore[:, t0 * BC:(t0 + EBLK) * BC], EXP,
                        bias=bc_s[:, 0:1]
                    )

                emit_block(0)
                emit_block(31)
                # W0 = Y_0 * exp(T[BOS,:]) ; V = veb * Y_last
                # chain state in CH[parity, chain, :]: hop r reads parity
                # (r-1)%2, writes r%2 — no in-place WAR; the fwd and bwd hop
                # multiplies merge into ONE strided-AP tensor_tensor per r
                Yp = Y[0:NB, :].rearrange("p (t b) -> p t b", b=BC)
                nc.vector.tensor_scalar_mul(CH[:, 0, 0, :], Y[0:NB, 0:BC],
                                            etb[:, 0:1])
                nc.vector.tensor_mul(CH[:, 1, 1, :], veb[:, :],
                                     Y[0:NB, (SL - 1) * BC:SL * BC])

                # small phase-C PSUM tensors: one bank per chain so the fwd
                # and bwd renorm pipelines don't false-serialize on a bank
                crfF = pc2.tile([NB, 3 * BC], F32, tag="crfF")
                crfB = pc2.tile([NB, 2 * BC], F32, tag="crfB")
                rf_ps = crfF[:, 0:BC]
                sf_ps = crfF[0:1, BC:2 * BC]
                dot_ps = crfF[0:1, 2 * BC:3 * BC]
                rb_ps = crfB[:, 0:BC]
                sb_ps = crfB[0:1, BC:2 * BC]

                def renorm_snap(w_sb, s_ps):
                    # s = ones19^T @ w (PE, off the recurrence chain)
                    nc.tensor.matmul(s_ps, ones19[:, :], w_sb[:, :],
                                     skip_group_check=True)

                def renorm_mid(s_ps, rec, recb, r_ps, lg, acc):
                    nc.vector.reciprocal(rec[:, :], s_ps)
                    nc.vector.tensor_copy(recb[:, :], rec[:, :])
                    nc.tensor.matmul(r_ps, one1x19[:, :], recb[:, :],
                                     skip_group_check=True)
                    nc.scalar.activation(lg[:, :], s_ps, LOG)
                    nc.vector.tensor_add(acc[:, :], acc[:, :], lg[:, :])

                wb_prev = None
                HALF = SL // 2
                for r in range(HALF):
                    p, q = r % 2, (r + 1) % 2  # dst / src parity
                    ty = SL - 2 - r  # next Y column for backward chain
                    # just-in-time Y production (exp only): low block kk+1 at
                    # r=16kk+6, high block 30-kk at r=16kk+11 (blocks 0, 31
                    # are fully produced before the loop)
                    kk, jj = divmod(r, RENORM)
                    if kk <= 14 and jj == 6:
                        emit_exp(kk + 1)
                    elif kk <= 14 and jj == 11:
                        emit_exp(30 - kk)
                    wfb_ps = pc.tile([NB, 2, BC], F32, tag="wfb_ps")
                    # backward chain mm (always)
                    nc.tensor.matmul(wfb_ps[:, 1, :], est[:, :],
                                     CH[:, q, 1, :], skip_group_check=True)
                    # forward chain mm: t = r = 1..HALF-1
                    if r >= 1:
                        nc.tensor.matmul(wfb_ps[:, 0, :], esm[:, :],
                                         CH[:, q, 0, :], skip_group_check=True)
                    if r % RENORM == 0 and r >= RENORM:
                        renorm_snap(CH[:, q, 0, :], sf_ps)
                        renorm_snap(CH[:, q, 1, :], sb_ps)
                    # hop multiplies: one strided TT covers both chains
                    if 1 <= r < HALF - 1:
                        nc.vector.tensor_mul(
                            CH[:, p, :, :], wfb_ps[:, :, :],
                            Yp[:, r:ty + 1:(ty - r), :])
                    elif r == 0:
                        nc.vector.tensor_mul(
                            CH[:, p, 1, :], wfb_ps[:, 1, :],
                            Y[0:NB, ty * BC:(ty + 1) * BC])
                    else:  # r == HALF-1: forward hop only
                        nc.vector.tensor_mul(
                            CH[:, p, 0, :], wfb_ps[:, 0, :],
                            Y[0:NB, r * BC:(r + 1) * BC])
                    if r % RENORM == 0 and r >= RENORM:
                        renorm_mid(sf_ps, rec_f, rec_fb, rf_ps, lg_f, acc_f)
                        renorm_mid(sb_ps, rec_b, rec_bb, rb_ps, lg_b, acc_b)
                    if r >= RENORM + 3 and (r - 3) % RENORM == 0:
                        nc.vector.tensor_mul(CH[:, p, 0, :], CH[:, p, 0, :],
                                             rf_ps)
                        nc.vector.tensor_mul(CH[:, p, 1, :], CH[:, p, 1, :],
                                             rb_ps)
                    wb_prev = wfb_ps

                # meet at t=HALF-1: P2 = Wf_{HALF-1} * beta_{HALF-1}
                nc.vector.tensor_mul(P2[:, :], CH[:, (HALF - 1) % 2, 0, :],
                                     wb_prev[:, 1, :])
                nc.tensor.matmul(dot_ps, ones19f[:, :], P2[:, :],
                                 skip_group_check=True)
                nc.scalar.activation(res_s[0:1, :], dot_ps, LOG)

            nc.sync.dma_start(out=y_out[:], in_=Y[:, :])
            nc.sync.dma_start(out=res_out[0:1], in_=res_s[0:1, :])
            nc.sync.dma_start(out=res_out[1:2], in_=acc_f[:, :])
            nc.sync.dma_start(out=res_out[2:3], in_=acc_b[:, :])

    return nc


def _split_waits(nc):
    """Walrus codegen allows ~1 sync-wait on compute instrs; move excess
    waits onto injected same-engine Drain instructions (which allow many).

    Keep the wait most likely to be satisfied LAST inline on the compute
    instruction (a cross-engine producer), and drain the early-satisfied
    ones (same-engine program-order waits) first — a drain blocked on the
    critical producer adds ~70-90ns of serial queue decode vs an inline
    wait that fires as soon as the semaphore lands."""
    from concourse import mybir as mb

    def sem_engine(w):
        nm = getattr(w, 'ant_name', '') or ''
        return nm.split('_')[0]

    eng_name = {
        mb.EngineType.PE: 'PE', mb.EngineType.Activation: 'Activation',
        mb.EngineType.DVE: 'DVE', mb.EngineType.Pool: 'Pool',
        mb.EngineType.SP: 'SP',
    }
    n = 0
    for f in nc.m.functions:
        for blk in f.blocks:
            insts = blk.instructions
            new_list = []
            for ins in insts:
                si = ins.sync_info
                if si is not None and si.on_wait and len(si.on_wait) > 1:
                    waits = list(si.on_wait)
                    own = eng_name.get(ins.engine, '?')
                    cross = [w for w in waits if sem_engine(w) != own]
                    selfw = [w for w in waits if sem_engine(w) == own]
                    inline = [cross[-1]] if cross else [waits[-1]]
                    rest = [w for w in waits if w is not inline[0]]
                    # self-engine waits first (satisfied early), cross after
                    rest.sort(key=lambda w: 0 if sem_engine(w) == own else 1)
                    for w in rest:
                        d = mb.InstDrain(
                            name=f"{ins.name}-ws{n}", ins=[], outs=[])
                        d.engine = ins.engine
                        d.sync_info = mb.SyncInfo(on_wait=[w], on_update=[])
                        new_list.append(d)
                        n += 1
                    ins.sync_info = mb.SyncInfo(
                        on_wait=inline, on_update=list(si.on_update))
                new_list.append(ins)
            del insts[:]
            insts.extend(new_list)
    return n


def _prep_host(inputs):
    emb = np.asarray(inputs["emb"], np.float32)
    T = np.asarray(inputs["transitions"], np.float32)
    W1 = np.asarray(inputs["W1"], np.float32)
    b1 = np.asarray(inputs["b1"], np.float32)
    W2 = np.asarray(inputs["W2"], np.float32)
    b2 = np.asarray(inputs["b2"], np.float32)

    emb_pad = np.zeros((V, EPAD), np.float32)
    emb_pad[:, 0:I] = emb
    emb_pad[:, I] = 1.0  # bias-aug ones row

    # gate reorder: pytorch [i,f,g,o] -> ours [f,i,g,o]
    perm = np.concatenate([np.arange(I, 2 * I), np.arange(0, I),
                           np.arange(2 * I, 3 * I), np.arange(3 * I, 4 * I)])

    def pack_dir(Wih, Whh, bih, bhh):
        Wih, Whh = Wih[perm].copy(), Whh[perm].copy()
        bias = (bih + bhh)[perm].copy()
        # 2x the g-gate pre-acts: kernel computes tanh(g) as 2*sigma(2g)-1
        Wih[2 * I:3 * I] *= 2.0
        Whh[2 * I:3 * I] *= 2.0
        bias[2 * I:3 * I] *= 2.0
        wih = np.zeros((4, 128, 128), np.float32)  # [gamma, k, m]
        whh = np.zeros((4, 128, 128), np.float32)
        for g in range(4):
            wih[g, 0:I, 0:I] = Wih[g * I:(g + 1) * I].T
            wih[g, I, 0:I] = bias[g * I:(g + 1) * I]
            whh[g, 0:I, 0:I] = Whh[g * I:(g + 1) * I].T
        return wih, whh

    wih_f, whh_f = pack_dir(np.asarray(inputs["Wih_f"], np.float32),
                            np.asarray(inputs["Whh_f"], np.float32),
                            np.asarray(inputs["bih_f"], np.float32),
                            np.asarray(inputs["bhh_f"], np.float32))
    wih_b, whh_b = pack_dir(np.asarray(inputs["Wih_b"], np.float32),
                            np.asarray(inputs["Whh_b"], np.float32),
                            np.asarray(inputs["bih_b"], np.float32),
                            np.asarray(inputs["bhh_b"], np.float32))

    wih = np.zeros((128, GCH, 128), np.float32)
    whh = np.zeros((128, GCH, 128), np.float32)
    for g in range(4):
        wih[:, g * 2 + 0, :] = wih_f[g]
        wih[:, g * 2 + 1, :] = wih_b[g]
        whh[:, g * 2 + 0, :] = whh_f[g]
        whh[:, g * 2 + 1, :] = whh_b[g]

    Wc = W2 @ W1                      # [19, 200]
    bcv = W2 @ b1 + b2                # [19]
    wc = np.zeros((128, 2, NB), np.float32)
    wc[0:I, 0, :] = Wc[:, 0:I].T
    wc[0:I, 1, :] = Wc[:, I:2 * I].T

    c0 = float(np.log(np.sum(np.exp(bcv))))
    esm = np.exp(T - c0)
    est = esm.T.copy()
    etb = np.exp(T[BOS, :]).reshape(NB, 1)
    veb = np.broadcast_to(np.exp(T[:, EOS]).reshape(NB, 1), (NB, BC)).copy()

    bf = ml_dtypes.bfloat16
    common = {
        "emb_pad": emb_pad.astype(bf),
        "wih": wih.astype(bf),
        "whh": whh.astype(bf),
        "wc": wc.astype(bf),
        "bc": bcv.reshape(NB, 1).astype(np.float32),
        "esm": esm.astype(bf),
        "est": est.astype(bf),
        "etb": etb.astype(np.float32),
        "veb": veb.astype(np.float32),
        "ones19": np.ones((NB, 1), bf),
        "ones19f": np.ones((NB, 1), np.float32),
        "one1x19": np.ones((1, NB), bf),
    }
    return common, c0, bcv


def kernel(**inputs):
    x = np.asarray(inputs["x"]).reshape(B, S).astype(np.int64)
    target = np.asarray(inputs["target"]).reshape(B, S).astype(np.int64)
    T = np.asarray(inputs["transitions"], np.float32)

    common, c0, bcv = _prep_host(inputs)

    common["eye"] = np.eye(128, dtype=ml_dtypes.bfloat16)
    in_maps = []
    for c in range(NCORES):
        xs = x[c * BC:(c + 1) * BC]  # [BC, S]
        # fwd token order: col t*BC + b  -> x[b, t]
        idx_fwd = xs.T.reshape(-1).astype(np.int32)
        idxs = idx_fwd.reshape(NT // 128, 128).T.copy()
        in_maps.append({**common, "idxs": idxs})

    if "nc" not in _CACHE:
        nc0 = _build_nc()
        _split_waits(nc0)
        mybir.codegen_inst_isa_subclasses(nc0)
        _CACHE["nc"] = nc0
    nc = _CACHE["nc"]
    _CACHE["in_maps"] = in_maps

    results = run_bass_kernel_spmd(nc, in_maps, list(range(NCORES))).results

    # host combine
    t_sc = (T[target[:, :-1], target[:, 1:]].sum(1)
            + T[BOS, target[:, 0]] + T[target[:, -1], EOS])  # [B]

    losses = np.zeros(B, np.float64)
    for c in range(NCORES):
        yv = np.asarray(results[c]["y_out"], ml_dtypes.bfloat16).astype(np.float32)
        res = np.asarray(results[c]["res"], np.float32)
        logY = np.log(yv).reshape(NB, S, BC)  # log Y = em + bc - c0... (em+bc)
        tg = target[c * BC:(c + 1) * BC]      # [BC, S]
        bi = np.arange(BC)
        e_sc = np.zeros(BC, np.float64)
        for t in range(S):
            e_sc += logY[tg[:, t], t, bi]
        partition = res[0] + res[1] + res[2] + (S - 1) * c0
        losses[c * BC:(c + 1) * BC] = (
            e_sc + t_sc[c * BC:(c + 1) * BC] - partition
        )
    return np.float32(-losses.mean())



# revision 39
# speedup vs baseline: 1.0940x; 1.0911x over previous
"""BiLSTM+CRF NLL loss kernel for 8 Trainium2 NeuronCores (v3).

Sharding: data-parallel on batch (32 sequences per core). Each core runs the
full BiLSTM + emission + CRF forward/backward partition recurrences for its
shard; host combines per-core partials into the scalar loss.

v3 vs v2 (trace: 2551ns/step chain = mm,σ,tanh_g,mul,add,tanh_c,mul_h):
- tanh(g) removed from the serial ACT chain: g pre-acts are scaled 2x on the
  host so one sigmoid instruction covers f,i,g (tanh(g) = 2σ(2g)-1), and the
  cell update becomes c = f*c + 2(σ_g-0.5)*i via fused scalar_tensor_tensor
- cell/gate DVE pipeline in bf16 (2x DVE rate), cell updated in place in a
  fixed tile (no cross-engine WAR)
- CRF fwd/bwd hop multiplies merged into one strided-AP tensor_tensor
"""

import numpy as np
import ml_dtypes

import concourse.bass as bass
import concourse.tile as tile
from concourse import mybir
from concourse.bass_utils import run_bass_kernel_spmd

F32 = mybir.dt.float32
BF16 = mybir.dt.bfloat16

B, S, V, I, NB = 256, 512, 30000, 100, 19
BOS, EOS = 17, 18
NCORES = 8
BC = B // NCORES          # 32 sequences per core
NT = BC * S               # 16384 tokens per core
KP = I + 1                # 101: embedding dims + ones row (bias aug)
EPAD = 128                # padded embedding row length
RENORM = 16               # CRF renorm interval
TBLK = 4                  # steps per PSUM gate block
GCH = 8                   # gate chunks: (gamma in [g,f,i,o]) x (dir in [f,b])

_CACHE = {}


def _build_nc(s_len=S):
    SL = s_len
    NTL = BC * SL
    NBLK = SL // TBLK
    NCHUNK = NTL // 128

    nc = bass.Bass()

    # ---- dram I/O ----
    # xet = host-pre-gathered, pre-transposed embeddings (bias-aug row
    # included): the on-device indirect gather + transpose machinery was the
    # phase A startup bottleneck (random 256B-row HBM gathers at ~3us/chunk)
    xet_d = nc.dram_tensor("xet", [128, NTL], BF16, kind="ExternalInput")
    wih_d = nc.dram_tensor("wih", [128, GCH, 128], BF16, kind="ExternalInput")
    whh_d = nc.dram_tensor("whh", [128, GCH, 128], BF16, kind="ExternalInput")
    wc_d = nc.dram_tensor("wc", [128, 2, NB], BF16, kind="ExternalInput")
    bc_d = nc.dram_tensor("bc", [NB, 1], F32, kind="ExternalInput")
    esm_d = nc.dram_tensor("esm", [NB, NB], BF16, kind="ExternalInput")
    est_d = nc.dram_tensor("est", [NB, NB], BF16, kind="ExternalInput")
    etb_d = nc.dram_tensor("etb", [NB, 1], F32, kind="ExternalInput")
    veb_d = nc.dram_tensor("veb", [NB, BC], F32, kind="ExternalInput")
    ones19_d = nc.dram_tensor("ones19", [NB, 1], BF16, kind="ExternalInput")
    ones19f_d = nc.dram_tensor("ones19f", [NB, 1], F32, kind="ExternalInput")
    one1x19_d = nc.dram_tensor("one1x19", [1, NB], BF16, kind="ExternalInput")

    y_out = nc.dram_tensor("y_out", [NB, NTL], BF16, kind="ExternalOutput")
    res_out = nc.dram_tensor("res", [4, BC], F32, kind="ExternalOutput")

    SIG = mybir.ActivationFunctionType.Sigmoid
    TANH = mybir.ActivationFunctionType.Tanh
    EXP = mybir.ActivationFunctionType.Exp
    LOG = mybir.ActivationFunctionType.Ln

    with tile.TileContext(nc) as tc:
        with tc.tile_pool(name="big", bufs=1) as bp:
            xeT_f = bp.tile([128, NTL], BF16, tag="xeT_f")
            emstore = bp.tile([NB, NTL], BF16, tag="emstore")
            # h storage: col (t+1)*32 = h after step t; col 0 = h(-1)=0
            h_all = bp.tile([128, 2, NTL + BC], BF16, tag="h_all")
            Y = bp.tile([NB, NTL], BF16, tag="Y")
            wih = bp.tile([128, GCH, 128], BF16, tag="wih")
            whh = bp.tile([128, GCH, 128], BF16, tag="whh")
            wc = bp.tile([128, 2, NB], BF16, tag="wc")
            bc_s = bp.tile([NB, 1], F32, tag="bc_s")
            esm = bp.tile([NB, NB], BF16, tag="esm")
            est = bp.tile([NB, NB], BF16, tag="est")
            etb = bp.tile([NB, 1], F32, tag="etb")
            veb = bp.tile([NB, BC], F32, tag="veb")
            ones19 = bp.tile([NB, 1], BF16, tag="ones19")
            ones19f = bp.tile([NB, 1], F32, tag="ones19f")
            one1x19 = bp.tile([1, NB], BF16, tag="one1x19")
            gates_s0 = bp.tile([128, GCH, BC], BF16, tag="gates_s0")
            gates_s1 = bp.tile([128, GCH, BC], BF16, tag="gates_s1")
            cellc = bp.tile([128, 2, BC], BF16, tag="cellc")  # c, in-place
            th0 = bp.tile([128, 2, BC], BF16, tag="th0")
            th1 = bp.tile([128, 2, BC], BF16, tag="th1")
            u_f = bp.tile([128, 2, BC], BF16, tag="u_f")
            u_i = bp.tile([128, 2, BC], BF16, tag="u_i")
            # CRF chain state: [parity, chain(0=fwd,1=bwd), BC]
            CH = bp.tile([NB, 2, 2, BC], BF16, tag="CH")
            P2 = bp.tile([NB, BC], F32, tag="P2")
            acc_f = bp.tile([1, BC], F32, tag="acc_f")
            acc_b = bp.tile([1, BC], F32, tag="acc_b")
            rec_f = bp.tile([1, BC], F32, tag="rec_f")
            rec_fb = bp.tile([1, BC], BF16, tag="rec_fb")
            rec_b = bp.tile([1, BC], F32, tag="rec_b")
            rec_bb = bp.tile([1, BC], BF16, tag="rec_bb")
            lg_f = bp.tile([1, BC], F32, tag="lg_f")
            lg_b = bp.tile([1, BC], F32, tag="lg_b")
            res_s = bp.tile([4, BC], F32, tag="res_s")

            # ---- loads: early xet ranges first (blocks 0.. and ..127),
            # ---- then weights, then the xet middle ----
            C16 = 16 * 128
            nc.sync.dma_start(out=xeT_f[:, 0:C16], in_=xet_d[:, 0:C16])
            nc.sync.dma_start(out=xeT_f[:, NTL - C16:NTL],
                              in_=xet_d[:, NTL - C16:NTL])
            nc.sync.dma_start(out=wih[:, :, :], in_=wih_d[:])
            nc.sync.dma_start(out=whh[:, :, :], in_=whh_d[:])
            nc.sync.dma_start(out=wc[:, :, :], in_=wc_d[:])
            nc.sync.dma_start(out=bc_s[:, :], in_=bc_d[:])
            nc.sync.dma_start(out=esm[:, :], in_=esm_d[:])
            nc.sync.dma_start(out=est[:, :], in_=est_d[:])
            nc.sync.dma_start(out=etb[:, :], in_=etb_d[:])
            nc.sync.dma_start(out=veb[:, :], in_=veb_d[:])
            nc.sync.dma_start(out=ones19[:, :], in_=ones19_d[:])
            nc.sync.dma_start(out=ones19f[:, :], in_=ones19f_d[:])
            nc.sync.dma_start(out=one1x19[:, :], in_=one1x19_d[:])
            # xet middle: two big DMAs, landing well before consumption
            nc.sync.dma_start(out=xeT_f[:, C16:NTL // 2],
                              in_=xet_d[:, C16:NTL // 2])
            nc.sync.dma_start(out=xeT_f[:, NTL // 2:NTL - C16],
                              in_=xet_d[:, NTL // 2:NTL - C16])

            nc.vector.memset(acc_f[:, :], 0.0)
            nc.vector.memset(acc_b[:, :], 0.0)

            # =========== phase A: both LSTMs, lockstep ==========
            with tc.tile_pool(name="psE", bufs=2, space="PSUM") as pe_pool, \
                 tc.tile_pool(name="psA", bufs=1, space="PSUM") as pa:
                GA = pa.tile([128, GCH, TBLK, BC], F32, tag="GA")
                GB = pa.tile([128, GCH, TBLK, BC], F32, tag="GB")
                gbuf = (GA, GB)

                xe3 = xeT_f[0:KP, :].rearrange("p (t b) -> p t b", b=BC)

                def bulk_mm(k, c):
                    G = gbuf[k % 2]
                    if c % 2 == 0:
                        rhs = xe3[:, k * TBLK:(k + 1) * TBLK, :]
                    else:
                        hi = SL - 1 - k * TBLK
                        rhs = (xe3[:, hi:hi - TBLK:-1, :]
                               if hi - TBLK >= 0 else xe3[:, hi::-1, :])
                    nc.tensor.matmul(
                        G[:, c, :, :], wih[0:KP, c, :], rhs,
                        start=True, stop=False, skip_group_check=True,
                    )

                MULT = mybir.AluOpType.mult
                ADD = mybir.AluOpType.add
                SUB = mybir.AluOpType.subtract

                def step(t):
                    G = gbuf[(t // TBLK) % 2]
                    tau = t % TBLK
                    rd = t * BC
                    gs = gates_s0 if t % 2 == 0 else gates_s1
                    th = th0 if t % 2 == 0 else th1
                    if t > 0:
                        # recurrent matmuls: f,i,g first, then o
                        for c in (0, 1, 2, 3, 4, 5):
                            d = c % 2
                            nc.tensor.matmul(
                                G[:, c, tau, :], whh[0:I, c, :],
                                h_all[0:I, d, rd:rd + BC],
                                start=False, stop=True, skip_group_check=True,
                            )
                    # one sigmoid covers f,i,g (g pre-acts carry a 2x host
                    # scale, so sigma here encodes tanh(g) = 2*sigma(2g)-1)
                    nc.scalar.activation(gs[:, 0:6, :], G[:, 0:6, tau, :], SIG)
                    if t > 0:
                        for c in (6, 7):
                            d = c % 2
                            nc.tensor.matmul(
                                G[:, c, tau, :], whh[0:I, c, :],
                                h_all[0:I, d, rd:rd + BC],
                                start=False, stop=True, skip_group_check=True,
                            )
                    # sigmoid(o) off the critical path
                    nc.scalar.activation(gs[:, 6:8, :], G[:, 6:8, tau, :], SIG)
                    # cellc tracks the HALF-cell d = c/2, making the cell
                    # update end in a plain add: d = f*d + (sigma_g-0.5)*i
                    # [= f*c/2 + tanh(g)/2*i]; tanh(c) = tanh(2d) via scale
                    nc.vector.scalar_tensor_tensor(
                        u_i[:, :, :], gs[:, 4:6, :], 0.5, gs[:, 2:4, :],
                        op0=SUB, op1=MULT)
                    if t > 0:
                        nc.vector.tensor_mul(
                            u_f[:, :, :], gs[:, 0:2, :], cellc[:, :, :])
                        nc.vector.tensor_add(
                            cellc[:, :, :], u_f[:, :, :], u_i[:, :, :])
                    else:
                        nc.vector.tensor_copy(cellc[:, :, :], u_i[:, :, :])
                    nc.scalar.activation(th[:, :, :], cellc[:, :, :], TANH,
                                         scale=2.0)
                    wr = (t + 1) * BC
                    nc.vector.tensor_mul(
                        h_all[:, :, wr:wr + BC], gs[:, 6:8, :], th[:, :, :]
                    )

                for c in range(GCH):
                    bulk_mm(0, c)
                for c in range(GCH):
                    bulk_mm(1, c)

                # in-A emission production: emission matmuls for the middle
                # blocks run here (h for block j is complete from iter
                # max(16j+15, 496-16j); PE has slack), storing pre-exp
                # emissions to SBUF in bf16. Phase C then only runs the EXPs
                # (no activation-table thrash: exp stays out of phase A).
                EBLK = 16
                HBq = 4  # tokens per emission quarter-matmul
                hb_ap_a = h_all[0:I, 1, :].rearrange("p (t b) -> p t b", b=BC)
                ema_tiles = {}

                def ema_quarter(blk, part):
                    t0 = blk * EBLK
                    if blk not in ema_tiles:
                        ema_new = pe_pool.tile([NB, EBLK * BC], F32,
                                               tag="ema_ps")
                        ema_tiles[blk] = ema_new
                    em_ps = ema_tiles[blk]
                    th0_ = t0 + part * HBq
                    sl = slice(part * HBq * BC, (part + 1) * HBq * BC)
                    # hf for token t lives at col (t+1)*BC
                    nc.tensor.matmul(
                        em_ps[:, sl], wc[0:I, 0, :],
                        h_all[0:I, 0, (th0_ + 1) * BC:(th0_ + 1 + HBq) * BC],
                        start=True, stop=False, skip_group_check=True,
                    )
                    # hb for token t lives at round (SL-1-t): col (SL-t)*BC
                    nc.tensor.matmul(
                        em_ps[:, sl].rearrange("p (t b) -> p t b", b=BC),
                        wc[0:I, 1, :],
                        hb_ap_a[:, SL - th0_:SL - th0_ - HBq:-1, :],
                        start=False, stop=True, skip_group_check=True,
                    )

                def ema_cast(blk, half):
                    # halves bound the DVE head-of-line delay to ~370ns
                    t0 = blk * EBLK
                    em_ps = ema_tiles[blk]
                    HC = EBLK * BC // 2
                    nc.vector.tensor_copy(
                        emstore[:, t0 * BC + half * HC:t0 * BC + (half + 1) * HC],
                        em_ps[:, half * HC:(half + 1) * HC])
                    if half == 1:
                        ema_tiles.pop(blk)

                def ema_slot(t):
                    # window k covers iters [264+16k, 280+16k) and produces
                    # blocks lo=15-k and hi=16+k. Per-part readiness (h_f[t]
                    # at iter t, h_b[t] at iter 511-t): lo part p ready at
                    # iter 271+16k-4p (reverse order), hi part p at 259+16k+4p
                    if not (264 <= t < 504):
                        return
                    kwin, loc = divmod(t - 264, 16)
                    lo, hi = 15 - kwin, 16 + kwin
                    if loc == 0:
                        ema_quarter(lo, 3)
                    elif loc == 1:
                        ema_quarter(lo, 2)
                    elif loc == 2:
                        ema_quarter(hi, 0)
                    elif loc == 5:
                        ema_quarter(lo, 1)
                    elif loc == 4:
                        ema_quarter(hi, 1)
                    elif loc == 6:
                        ema_quarter(hi, 2)
                    elif loc == 8:
                        ema_quarter(lo, 0)
                    elif loc == 9:
                        ema_quarter(hi, 3)
                    elif loc == 10:
                        ema_cast(lo, 0)
                    elif loc == 11:
                        ema_cast(lo, 1)
                    elif loc == 12:
                        ema_cast(hi, 0)
                    elif loc == 13:
                        ema_cast(hi, 1)

                # chunk c serves fwd block c and bwd block NCHUNK-1-c, so every
                # chunk must be resident before block NBLK//2. DMA the chunk
                for k in range(NBLK):
                    for tau in range(TBLK):
                        step(k * TBLK + tau)
                        ema_slot(k * TBLK + tau)
                        # spread bulk (x-part) matmuls across the block
                        if k + 2 < NBLK:
                            if tau == 0:
                                bulk_mm(k + 2, 0)
                                bulk_mm(k + 2, 2)
                            elif tau == 1:
                                bulk_mm(k + 2, 4)
                                bulk_mm(k + 2, 6)
                            elif tau == 2:
                                bulk_mm(k + 2, 1)
                                bulk_mm(k + 2, 3)
                            else:
                                bulk_mm(k + 2, 5)
                                bulk_mm(k + 2, 7)

            # ==== phase B+C: Y = exp(em + bc) from the in-A emission store,
            # ==== interleaved with the CRF partition chains (exp and ln
            # ==== share the natural_log_exp activation table). Only blocks
            # ==== 0 and 31 (which need the very last LSTM iters) run their
            # ==== matmuls here.
            with tc.tile_pool(name="psB", bufs=2, space="PSUM") as pb, \
                 tc.tile_pool(name="psC", bufs=2, space="PSUM") as pc, \
                 tc.tile_pool(name="psC2", bufs=1, space="PSUM") as pc2:

                def emit_block(blk):
                    t0 = blk * EBLK
                    em_new = pb.tile([NB, EBLK * BC], F32, tag="em_ps")
                    for part in range(4):
                        th0_ = t0 + part * HBq
                        sl = slice(part * HBq * BC, (part + 1) * HBq * BC)
                        nc.tensor.matmul(
                            em_new[:, sl], wc[0:I, 0, :],
                            h_all[0:I, 0,
                                  (th0_ + 1) * BC:(th0_ + 1 + HBq) * BC],
                            start=True, stop=False, skip_group_check=True,
                        )
                        nc.tensor.matmul(
                            em_new[:, sl].rearrange("p (t b) -> p t b", b=BC),
                            wc[0:I, 1, :],
                            hb_ap_a[:, SL - th0_:SL - th0_ - HBq:-1, :],
                            start=False, stop=True, skip_group_check=True,
                        )
                    nc.scalar.activation(
                        Y[:, t0 * BC:(t0 + EBLK) * BC], em_new[:, :], EXP,
                        bias=bc_s[:, 0:1]
                    )

                def emit_exp(blk):
                    # exp over the bf16 pre-emissions stored during phase A
                    t0 = blk * EBLK
                    nc.scalar.activation(
                        Y[:, t0 * BC:(t0 + EBLK) * BC],
                        emstore[:, t0 * BC:(t0 + EBLK) * BC], EXP,
                        bias=bc_s[:, 0:1]
                    )

                emit_block(0)
                emit_block(31)
                # W0 = Y_0 * exp(T[BOS,:]) ; V = veb * Y_last
                # chain state in CH[parity, chain, :]: hop r reads parity
                # (r-1)%2, writes r%2 — no in-place WAR; the fwd and bwd hop
                # multiplies merge into ONE strided-AP tensor_tensor per r
                Yp = Y[0:NB, :].rearrange("p (t b) -> p t b", b=BC)
                nc.vector.tensor_scalar_mul(CH[:, 0, 0, :], Y[0:NB, 0:BC],
                                            etb[:, 0:1])
                nc.vector.tensor_mul(CH[:, 1, 1, :], veb[:, :],
                                     Y[0:NB, (SL - 1) * BC:SL * BC])

                # small phase-C PSUM tensors: one bank per chain so the fwd
                # and bwd renorm pipelines don't false-serialize on a bank
                crfF = pc2.tile([NB, 3 * BC], F32, tag="crfF")
                crfB = pc2.tile([NB, 2 * BC], F32, tag="crfB")
                rf_ps = crfF[:, 0:BC]
                sf_ps = crfF[0:1, BC:2 * BC]
                dot_ps = crfF[0:1, 2 * BC:3 * BC]
                rb_ps = crfB[:, 0:BC]
                sb_ps = crfB[0:1, BC:2 * BC]

                def renorm_snap(w_sb, s_ps):
                    # s = ones19^T @ w (PE, off the recurrence chain)
                    nc.tensor.matmul(s_ps, ones19[:, :], w_sb[:, :],
                                     skip_group_check=True)

                def renorm_mid(s_ps, rec, recb, r_ps, lg, acc):
                    nc.vector.reciprocal(rec[:, :], s_ps)
                    nc.vector.tensor_copy(recb[:, :], rec[:, :])
                    nc.tensor.matmul(r_ps, one1x19[:, :], recb[:, :],
                                     skip_group_check=True)
                    nc.scalar.activation(lg[:, :], s_ps, LOG)
                    nc.vector.tensor_add(acc[:, :], acc[:, :], lg[:, :])

                wb_prev = None
                HALF = SL // 2
                for r in range(HALF):
                    p, q = r % 2, (r + 1) % 2  # dst / src parity
                    ty = SL - 2 - r  # next Y column for backward chain
                    # just-in-time Y production (exp only): low block kk+1 at
                    # r=16kk+6, high block 30-kk at r=16kk+11 (blocks 0, 31
                    # are fully produced before the loop)
                    kk, jj = divmod(r, RENORM)
                    if kk <= 14 and jj == 6:
                        emit_exp(kk + 1)
                    elif kk <= 14 and jj == 11:
                        emit_exp(30 - kk)
                    wfb_ps = pc.tile([NB, 2, BC], F32, tag="wfb_ps")
                    # backward chain mm (always)
                    nc.tensor.matmul(wfb_ps[:, 1, :], est[:, :],
                                     CH[:, q, 1, :], skip_group_check=True)
                    # forward chain mm: t = r = 1..HALF-1
                    if r >= 1:
                        nc.tensor.matmul(wfb_ps[:, 0, :], esm[:, :],
                                         CH[:, q, 0, :], skip_group_check=True)
                    if r % RENORM == 0 and r >= RENORM:
                        renorm_snap(CH[:, q, 0, :], sf_ps)
                        renorm_snap(CH[:, q, 1, :], sb_ps)
                    # hop multiplies: one strided TT covers both chains
                    if 1 <= r < HALF - 1:
                        nc.vector.tensor_mul(
                            CH[:, p, :, :], wfb_ps[:, :, :],
                            Yp[:, r:ty + 1:(ty - r), :])
                    elif r == 0:
                        nc.vector.tensor_mul(
                            CH[:, p, 1, :], wfb_ps[:, 1, :],
                            Y[0:NB, ty * BC:(ty + 1) * BC])
                    else:  # r == HALF-1: forward hop only
                        nc.vector.tensor_mul(
                            CH[:, p, 0, :], wfb_ps[:, 0, :],
                            Y[0:NB, r * BC:(r + 1) * BC])
                    if r % RENORM == 0 and r >= RENORM:
                        renorm_mid(sf_ps, rec_f, rec_fb, rf_ps, lg_f, acc_f)
                        renorm_mid(sb_ps, rec_b, rec_bb, rb_ps, lg_b, acc_b)
                    if r >= RENORM + 3 and (r - 3) % RENORM == 0:
                        nc.vector.tensor_mul(CH[:, p, 0, :], CH[:, p, 0, :],
                                             rf_ps)
                        nc.vector.tensor_mul(CH[:, p, 1, :], CH[:, p, 1, :],
                                             rb_ps)
                    wb_prev = wfb_ps

                # meet at t=HALF-1: P2 = Wf_{HALF-1} * beta_{HALF-1}
                nc.vector.tensor_mul(P2[:, :], CH[:, (HALF - 1) % 2, 0, :],
                                     wb_prev[:, 1, :])
                nc.tensor.matmul(dot_ps, ones19f[:, :], P2[:, :],
                                 skip_group_check=True)
                nc.scalar.activation(res_s[0:1, :], dot_ps, LOG)

            nc.sync.dma_start(out=y_out[:], in_=Y[:, :])
            nc.sync.dma_start(out=res_out[0:1], in_=res_s[0:1, :])
            nc.sync.dma_start(out=res_out[1:2], in_=acc_f[:, :])
            nc.sync.dma_start(out=res_out[2:3], in_=acc_b[:, :])

    return nc


def _split_waits(nc):
    """Walrus codegen allows ~1 sync-wait on compute instrs; move excess
    waits onto injected same-engine Drain instructions (which allow many).

    Keep the wait most likely to be satisfied LAST inline on the compute
    instruction (a cross-engine producer), and drain the early-satisfied
    ones (same-engine program-order waits) first — a drain blocked on the
    critical producer adds ~70-90ns of serial queue decode vs an inline
    wait that fires as soon as the semaphore lands."""
    from concourse import mybir as mb

    def sem_engine(w):
        nm = getattr(w, 'ant_name', '') or ''
        return nm.split('_')[0]

    eng_name = {
        mb.EngineType.PE: 'PE', mb.EngineType.Activation: 'Activation',
        mb.EngineType.DVE: 'DVE', mb.EngineType.Pool: 'Pool',
        mb.EngineType.SP: 'SP',
    }
    n = 0
    for f in nc.m.functions:
        for blk in f.blocks:
            insts = blk.instructions
            new_list = []
            for ins in insts:
                si = ins.sync_info
                if si is not None and si.on_wait and len(si.on_wait) > 1:
                    waits = list(si.on_wait)
                    own = eng_name.get(ins.engine, '?')
                    cross = [w for w in waits if sem_engine(w) != own]
                    selfw = [w for w in waits if sem_engine(w) == own]
                    inline = [cross[-1]] if cross else [waits[-1]]
                    rest = [w for w in waits if w is not inline[0]]
                    # self-engine waits first (satisfied early), cross after
                    rest.sort(key=lambda w: 0 if sem_engine(w) == own else 1)
                    for w in rest:
                        d = mb.InstDrain(
                            name=f"{ins.name}-ws{n}", ins=[], outs=[])
                        d.engine = ins.engine
                        d.sync_info = mb.SyncInfo(on_wait=[w], on_update=[])
                        new_list.append(d)
                        n += 1
                    ins.sync_info = mb.SyncInfo(
                        on_wait=inline, on_update=list(si.on_update))
                new_list.append(ins)
            del insts[:]
            insts.extend(new_list)
    return n


def _prep_host(inputs):
    emb = np.asarray(inputs["emb"], np.float32)
    T = np.asarray(inputs["transitions"], np.float32)
    W1 = np.asarray(inputs["W1"], np.float32)
    b1 = np.asarray(inputs["b1"], np.float32)
    W2 = np.asarray(inputs["W2"], np.float32)
    b2 = np.asarray(inputs["b2"], np.float32)

    emb_pad = np.zeros((V, EPAD), np.float32)
    emb_pad[:, 0:I] = emb
    emb_pad[:, I] = 1.0  # bias-aug ones row

    # gate reorder: pytorch [i,f,g,o] -> ours [f,i,g,o]
    perm = np.concatenate([np.arange(I, 2 * I), np.arange(0, I),
                           np.arange(2 * I, 3 * I), np.arange(3 * I, 4 * I)])

    def pack_dir(Wih, Whh, bih, bhh):
        Wih, Whh = Wih[perm].copy(), Whh[perm].copy()
        bias = (bih + bhh)[perm].copy()
        # 2x the g-gate pre-acts: kernel computes tanh(g) as 2*sigma(2g)-1
        Wih[2 * I:3 * I] *= 2.0
        Whh[2 * I:3 * I] *= 2.0
        bias[2 * I:3 * I] *= 2.0
        wih = np.zeros((4, 128, 128), np.float32)  # [gamma, k, m]
        whh = np.zeros((4, 128, 128), np.float32)
        for g in range(4):
            wih[g, 0:I, 0:I] = Wih[g * I:(g + 1) * I].T
            wih[g, I, 0:I] = bias[g * I:(g + 1) * I]
            whh[g, 0:I, 0:I] = Whh[g * I:(g + 1) * I].T
        return wih, whh

    wih_f, whh_f = pack_dir(np.asarray(inputs["Wih_f"], np.float32),
                            np.asarray(inputs["Whh_f"], np.float32),
                            np.asarray(inputs["bih_f"], np.float32),
                            np.asarray(inputs["bhh_f"], np.float32))
    wih_b, whh_b = pack_dir(np.asarray(inputs["Wih_b"], np.float32),
                            np.asarray(inputs["Whh_b"], np.float32),
                            np.asarray(inputs["bih_b"], np.float32),
                            np.asarray(inputs["bhh_b"], np.float32))

    wih = np.zeros((128, GCH, 128), np.float32)
    whh = np.zeros((128, GCH, 128), np.float32)
    for g in range(4):
        wih[:, g * 2 + 0, :] = wih_f[g]
        wih[:, g * 2 + 1, :] = wih_b[g]
        whh[:, g * 2 + 0, :] = whh_f[g]
        whh[:, g * 2 + 1, :] = whh_b[g]

    Wc = W2 @ W1                      # [19, 200]
    bcv = W2 @ b1 + b2                # [19]
    wc = np.zeros((128, 2, NB), np.float32)
    wc[0:I, 0, :] = Wc[:, 0:I].T
    wc[0:I, 1, :] = Wc[:, I:2 * I].T

    c0 = float(np.log(np.sum(np.exp(bcv))))
    esm = np.exp(T - c0)
    est = esm.T.copy()
    etb = np.exp(T[BOS, :]).reshape(NB, 1)
    veb = np.broadcast_to(np.exp(T[:, EOS]).reshape(NB, 1), (NB, BC)).copy()

    bf = ml_dtypes.bfloat16
    common = {
        "_emb_bf": emb_pad.astype(bf),  # host-side only (xet pre-gather)
        "wih": wih.astype(bf),
        "whh": whh.astype(bf),
        "wc": wc.astype(bf),
        "bc": bcv.reshape(NB, 1).astype(np.float32),
        "esm": esm.astype(bf),
        "est": est.astype(bf),
        "etb": etb.astype(np.float32),
        "veb": veb.astype(np.float32),
        "ones19": np.ones((NB, 1), bf),
        "ones19f": np.ones((NB, 1), np.float32),
        "one1x19": np.ones((1, NB), bf),
    }
    return common, c0, bcv


def kernel(**inputs):
    x = np.asarray(inputs["x"]).reshape(B, S).astype(np.int64)
    target = np.asarray(inputs["target"]).reshape(B, S).astype(np.int64)
    T = np.asarray(inputs["transitions"], np.float32)

    common, c0, bcv = _prep_host(inputs)

    emb_bf = common.pop("_emb_bf")
    in_maps = []
    for c in range(NCORES):
        xs = x[c * BC:(c + 1) * BC]  # [BC, S]
        # fwd token order: col t*BC + b  -> x[b, t]; host pre-gathers the
        # (bias-augmented) embeddings pre-transposed: xet[:, t*BC+b]
        idx_fwd = xs.T.reshape(-1)
        xet = np.ascontiguousarray(emb_bf[idx_fwd].T)
        in_maps.append({**common, "xet": xet})

    if "nc" not in _CACHE:
        nc0 = _build_nc()
        _split_waits(nc0)
        mybir.codegen_inst_isa_subclasses(nc0)
        _CACHE["nc"] = nc0
    nc = _CACHE["nc"]
    _CACHE["in_maps"] = in_maps

    results = run_bass_kernel_spmd(nc, in_maps, list(range(NCORES))).results

    # host combine
    t_sc = (T[target[:, :-1], target[:, 1:]].sum(1)
            + T[BOS, target[:, 0]] + T[target[:, -1], EOS])  # [B]

    losses = np.zeros(B, np.float64)
    for c in range(NCORES):
        yv = np.asarray(results[c]["y_out"], ml_dtypes.bfloat16).astype(np.float32)
        res = np.asarray(results[c]["res"], np.float32)
        logY = np.log(yv).reshape(NB, S, BC)  # log Y = em + bc - c0... (em+bc)
        tg = target[c * BC:(c + 1) * BC]      # [BC, S]
        bi = np.arange(BC)
        e_sc = np.zeros(BC, np.float64)
        for t in range(S):
            e_sc += logY[tg[:, t], t, bi]
        partition = res[0] + res[1] + res[2] + (S - 1) * c0
        losses[c * BC:(c + 1) * BC] = (
            e_sc + t_sc[c * BC:(c + 1) * BC] - partition
        )
    return np.float32(-losses.mean())



# revision 44
# speedup vs baseline: 1.1112x; 1.0158x over previous
"""BiLSTM+CRF NLL loss kernel for 8 Trainium2 NeuronCores (v3).

Sharding: data-parallel on batch (32 sequences per core). Each core runs the
full BiLSTM + emission + CRF forward/backward partition recurrences for its
shard; host combines per-core partials into the scalar loss.

v3 vs v2 (trace: 2551ns/step chain = mm,σ,tanh_g,mul,add,tanh_c,mul_h):
- tanh(g) removed from the serial ACT chain: g pre-acts are scaled 2x on the
  host so one sigmoid instruction covers f,i,g (tanh(g) = 2σ(2g)-1), and the
  cell update becomes c = f*c + 2(σ_g-0.5)*i via fused scalar_tensor_tensor
- cell/gate DVE pipeline in bf16 (2x DVE rate), cell updated in place in a
  fixed tile (no cross-engine WAR)
- CRF fwd/bwd hop multiplies merged into one strided-AP tensor_tensor
"""

import numpy as np
import ml_dtypes

import concourse.bass as bass
import concourse.tile as tile
from concourse import mybir
from concourse.bass_utils import run_bass_kernel_spmd

F32 = mybir.dt.float32
BF16 = mybir.dt.bfloat16

B, S, V, I, NB = 256, 512, 30000, 100, 19
BOS, EOS = 17, 18
NCORES = 8
BC = B // NCORES          # 32 sequences per core
NT = BC * S               # 16384 tokens per core
KP = I + 1                # 101: embedding dims + ones row (bias aug)
EPAD = 128                # padded embedding row length
RENORM = 16               # CRF renorm interval
TBLK = 4                  # steps per PSUM gate block
GCH = 8                   # gate chunks: (gamma in [g,f,i,o]) x (dir in [f,b])

_CACHE = {}


def _build_nc(s_len=S):
    SL = s_len
    NTL = BC * SL
    NBLK = SL // TBLK
    NCHUNK = NTL // 128

    nc = bass.Bass()

    # ---- dram I/O ----
    # xet = host-pre-gathered, pre-transposed embeddings (bias-aug row
    # included): the on-device indirect gather + transpose machinery was the
    # phase A startup bottleneck (random 256B-row HBM gathers at ~3us/chunk)
    xet_d = nc.dram_tensor("xet", [128, NTL], BF16, kind="ExternalInput")
    wih_d = nc.dram_tensor("wih", [128, GCH, 128], BF16, kind="ExternalInput")
    whh_d = nc.dram_tensor("whh", [128, GCH, 128], BF16, kind="ExternalInput")
    wc_d = nc.dram_tensor("wc", [128, 2, NB], BF16, kind="ExternalInput")
    bc_d = nc.dram_tensor("bc", [NB, 1], F32, kind="ExternalInput")
    esm_d = nc.dram_tensor("esm", [NB, NB], BF16, kind="ExternalInput")
    est_d = nc.dram_tensor("est", [NB, NB], BF16, kind="ExternalInput")
    etb_d = nc.dram_tensor("etb", [NB, 1], F32, kind="ExternalInput")
    veb_d = nc.dram_tensor("veb", [NB, BC], F32, kind="ExternalInput")
    ones19_d = nc.dram_tensor("ones19", [NB, 1], BF16, kind="ExternalInput")
    ones19f_d = nc.dram_tensor("ones19f", [NB, 1], F32, kind="ExternalInput")
    one1x19_d = nc.dram_tensor("one1x19", [1, NB], BF16, kind="ExternalInput")

    y_out = nc.dram_tensor("y_out", [NB, NTL], BF16, kind="ExternalOutput")
    res_out = nc.dram_tensor("res", [4, BC], F32, kind="ExternalOutput")

    SIG = mybir.ActivationFunctionType.Sigmoid
    TANH = mybir.ActivationFunctionType.Tanh
    EXP = mybir.ActivationFunctionType.Exp
    LOG = mybir.ActivationFunctionType.Ln

    with tile.TileContext(nc) as tc:
        with tc.tile_pool(name="big", bufs=1) as bp:
            xeT_f = bp.tile([128, NTL], BF16, tag="xeT_f")
            emstore = bp.tile([NB, NTL], BF16, tag="emstore")
            # h storage: col (t+1)*32 = h after step t; col 0 = h(-1)=0
            h_all = bp.tile([128, 2, NTL + BC], BF16, tag="h_all")
            Y = bp.tile([NB, NTL], BF16, tag="Y")
            wih = bp.tile([128, GCH, 128], BF16, tag="wih")
            whh = bp.tile([128, GCH, 128], BF16, tag="whh")
            wc = bp.tile([128, 2, NB], BF16, tag="wc")
            bc_s = bp.tile([NB, 1], F32, tag="bc_s")
            esm = bp.tile([NB, NB], BF16, tag="esm")
            est = bp.tile([NB, NB], BF16, tag="est")
            etb = bp.tile([NB, 1], F32, tag="etb")
            veb = bp.tile([NB, BC], F32, tag="veb")
            ones19 = bp.tile([NB, 1], BF16, tag="ones19")
            ones19f = bp.tile([NB, 1], F32, tag="ones19f")
            one1x19 = bp.tile([1, NB], BF16, tag="one1x19")
            gates_s0 = bp.tile([128, GCH, BC], BF16, tag="gates_s0")
            gates_s1 = bp.tile([128, GCH, BC], BF16, tag="gates_s1")
            cellc = bp.tile([128, 2, BC], BF16, tag="cellc")  # c, in-place
            th0 = bp.tile([128, 2, BC], BF16, tag="th0")
            th1 = bp.tile([128, 2, BC], BF16, tag="th1")
            u_f = bp.tile([128, 2, BC], BF16, tag="u_f")
            u_i = bp.tile([128, 2, BC], BF16, tag="u_i")
            # CRF chain state: [parity, chain(0=fwd,1=bwd), BC]
            CH = bp.tile([NB, 2, 2, BC], BF16, tag="CH")
            P2 = bp.tile([NB, BC], F32, tag="P2")
            acc_f = bp.tile([1, BC], F32, tag="acc_f")
            acc_b = bp.tile([1, BC], F32, tag="acc_b")
            bcp0 = bp.tile([NB, 1], F32, tag="bcp0")
            bcp1 = bp.tile([NB, 1], F32, tag="bcp1")
            rec_f = bp.tile([1, BC], F32, tag="rec_f")
            rec_fb = bp.tile([1, BC], BF16, tag="rec_fb")
            rec_b = bp.tile([1, BC], F32, tag="rec_b")
            rec_bb = bp.tile([1, BC], BF16, tag="rec_bb")
            lg_f = bp.tile([1, BC], F32, tag="lg_f")
            lg_b = bp.tile([1, BC], F32, tag="lg_b")
            res_s = bp.tile([4, BC], F32, tag="res_s")

            # ---- loads: early xet ranges first (blocks 0.. and ..127),
            # ---- then weights, then the xet middle ----
            C16 = 16 * 128
            nc.sync.dma_start(out=xeT_f[:, 0:C16], in_=xet_d[:, 0:C16])
            nc.sync.dma_start(out=xeT_f[:, NTL - C16:NTL],
                              in_=xet_d[:, NTL - C16:NTL])
            nc.sync.dma_start(out=wih[:, :, :], in_=wih_d[:])
            nc.sync.dma_start(out=whh[:, :, :], in_=whh_d[:])
            nc.sync.dma_start(out=wc[:, :, :], in_=wc_d[:])
            nc.sync.dma_start(out=bc_s[:, :], in_=bc_d[:])
            nc.sync.dma_start(out=esm[:, :], in_=esm_d[:])
            nc.sync.dma_start(out=est[:, :], in_=est_d[:])
            nc.sync.dma_start(out=etb[:, :], in_=etb_d[:])
            nc.sync.dma_start(out=veb[:, :], in_=veb_d[:])
            nc.sync.dma_start(out=ones19[:, :], in_=ones19_d[:])
            nc.sync.dma_start(out=ones19f[:, :], in_=ones19f_d[:])
            nc.sync.dma_start(out=one1x19[:, :], in_=one1x19_d[:])
            # xet middle: two big DMAs, landing well before consumption
            nc.sync.dma_start(out=xeT_f[:, C16:NTL // 2],
                              in_=xet_d[:, C16:NTL // 2])
            nc.sync.dma_start(out=xeT_f[:, NTL // 2:NTL - C16],
                              in_=xet_d[:, NTL // 2:NTL - C16])

            nc.vector.memset(acc_f[:, :], 0.0)
            nc.vector.memset(acc_b[:, :], 0.0)

            # =========== phase A: both LSTMs, lockstep ==========
            with tc.tile_pool(name="psE", bufs=2, space="PSUM") as pe_pool, \
                 tc.tile_pool(name="psA", bufs=1, space="PSUM") as pa:
                GA = pa.tile([128, GCH, TBLK, BC], F32, tag="GA")
                GB = pa.tile([128, GCH, TBLK, BC], F32, tag="GB")
                gbuf = (GA, GB)

                xe3 = xeT_f[0:KP, :].rearrange("p (t b) -> p t b", b=BC)

                def bulk_mm(k, c):
                    G = gbuf[k % 2]
                    if c % 2 == 0:
                        rhs = xe3[:, k * TBLK:(k + 1) * TBLK, :]
                    else:
                        hi = SL - 1 - k * TBLK
                        rhs = (xe3[:, hi:hi - TBLK:-1, :]
                               if hi - TBLK >= 0 else xe3[:, hi::-1, :])
                    nc.tensor.matmul(
                        G[:, c, :, :], wih[0:KP, c, :], rhs,
                        start=True, stop=False, skip_group_check=True,
                    )

                MULT = mybir.AluOpType.mult
                ADD = mybir.AluOpType.add
                SUB = mybir.AluOpType.subtract

                def step(t):
                    G = gbuf[(t // TBLK) % 2]
                    tau = t % TBLK
                    rd = t * BC
                    gs = gates_s0 if t % 2 == 0 else gates_s1
                    th = th0 if t % 2 == 0 else th1
                    if t > 0:
                        # recurrent matmuls: f,i,g first, then o
                        for c in (0, 1, 2, 3, 4, 5):
                            d = c % 2
                            nc.tensor.matmul(
                                G[:, c, tau, :], whh[0:I, c, :],
                                h_all[0:I, d, rd:rd + BC],
                                start=False, stop=True, skip_group_check=True,
                            )
                    # one sigmoid covers f,i,g (g pre-acts carry a 2x host
                    # scale, so sigma here encodes tanh(g) = 2*sigma(2g)-1)
                    nc.scalar.activation(gs[:, 0:6, :], G[:, 0:6, tau, :], SIG)
                    if t > 0:
                        for c in (6, 7):
                            d = c % 2
                            nc.tensor.matmul(
                                G[:, c, tau, :], whh[0:I, c, :],
                                h_all[0:I, d, rd:rd + BC],
                                start=False, stop=True, skip_group_check=True,
                            )
                    # sigmoid(o) off the critical path
                    nc.scalar.activation(gs[:, 6:8, :], G[:, 6:8, tau, :], SIG)
                    # cellc tracks the HALF-cell d = c/2, making the cell
                    # update end in a plain add: d = f*d + (sigma_g-0.5)*i
                    # [= f*c/2 + tanh(g)/2*i]; tanh(c) = tanh(2d) via scale
                    nc.vector.scalar_tensor_tensor(
                        u_i[:, :, :], gs[:, 4:6, :], 0.5, gs[:, 2:4, :],
                        op0=SUB, op1=MULT)
                    if t > 0:
                        nc.vector.tensor_mul(
                            u_f[:, :, :], gs[:, 0:2, :], cellc[:, :, :])
                        nc.vector.tensor_add(
                            cellc[:, :, :], u_f[:, :, :], u_i[:, :, :])
                    else:
                        nc.vector.tensor_copy(cellc[:, :, :], u_i[:, :, :])
                    nc.scalar.activation(th[:, :, :], cellc[:, :, :], TANH,
                                         scale=2.0)
                    wr = (t + 1) * BC
                    nc.vector.tensor_mul(
                        h_all[:, :, wr:wr + BC], gs[:, 6:8, :], th[:, :, :]
                    )

                for c in range(GCH):
                    bulk_mm(0, c)
                for c in range(GCH):
                    bulk_mm(1, c)

                # in-A emission production: emission matmuls for the middle
                # blocks run here (h for block j is complete from iter
                # max(16j+15, 496-16j); PE has slack), storing pre-exp
                # emissions to SBUF in bf16. Phase C then only runs the EXPs
                # (no activation-table thrash: exp stays out of phase A).
                EBLK = 16
                HBq = 4  # tokens per emission quarter-matmul
                hb_ap_a = h_all[0:I, 1, :].rearrange("p (t b) -> p t b", b=BC)
                ema_tiles = {}

                def ema_quarter(blk, part):
                    t0 = blk * EBLK
                    if blk not in ema_tiles:
                        ema_new = pe_pool.tile([NB, EBLK * BC], F32,
                                               tag="ema_ps")
                        ema_tiles[blk] = ema_new
                    em_ps = ema_tiles[blk]
                    th0_ = t0 + part * HBq
                    sl = slice(part * HBq * BC, (part + 1) * HBq * BC)
                    # hf for token t lives at col (t+1)*BC
                    nc.tensor.matmul(
                        em_ps[:, sl], wc[0:I, 0, :],
                        h_all[0:I, 0, (th0_ + 1) * BC:(th0_ + 1 + HBq) * BC],
                        start=True, stop=False, skip_group_check=True,
                    )
                    # hb for token t lives at round (SL-1-t): col (SL-t)*BC
                    nc.tensor.matmul(
                        em_ps[:, sl].rearrange("p (t b) -> p t b", b=BC),
                        wc[0:I, 1, :],
                        hb_ap_a[:, SL - th0_:SL - th0_ - HBq:-1, :],
                        start=False, stop=True, skip_group_check=True,
                    )

                def ema_cast(blk, half):
                    # halves bound the DVE head-of-line delay to ~370ns
                    t0 = blk * EBLK
                    em_ps = ema_tiles[blk]
                    HC = EBLK * BC // 2
                    nc.vector.tensor_copy(
                        emstore[:, t0 * BC + half * HC:t0 * BC + (half + 1) * HC],
                        em_ps[:, half * HC:(half + 1) * HC])
                    if half == 1:
                        ema_tiles.pop(blk)

                def ema_slot(t):
                    # window k covers iters [264+16k, 280+16k) and produces
                    # blocks lo=15-k and hi=16+k. Per-part readiness (h_f[t]
                    # at iter t, h_b[t] at iter 511-t): lo part p ready at
                    # iter 271+16k-4p (reverse order), hi part p at 259+16k+4p
                    if not (264 <= t < 504):
                        return
                    kwin, loc = divmod(t - 264, 16)
                    lo, hi = 15 - kwin, 16 + kwin
                    if loc == 0:
                        ema_quarter(lo, 3)
                    elif loc == 1:
                        ema_quarter(lo, 2)
                    elif loc == 2:
                        ema_quarter(hi, 0)
                    elif loc == 5:
                        ema_quarter(lo, 1)
                    elif loc == 4:
                        ema_quarter(hi, 1)
                    elif loc == 6:
                        ema_quarter(hi, 2)
                    elif loc == 8:
                        ema_quarter(lo, 0)
                    elif loc == 9:
                        ema_quarter(hi, 3)
                    elif loc == 10:
                        ema_cast(lo, 0)
                    elif loc == 11:
                        ema_cast(lo, 1)
                    elif loc == 12:
                        ema_cast(hi, 0)
                    elif loc == 13:
                        ema_cast(hi, 1)

                # chunk c serves fwd block c and bwd block NCHUNK-1-c, so every
                # chunk must be resident before block NBLK//2. DMA the chunk
                for k in range(NBLK):
                    for tau in range(TBLK):
                        step(k * TBLK + tau)
                        ema_slot(k * TBLK + tau)
                        # spread bulk (x-part) matmuls across the block
                        if k + 2 < NBLK:
                            if tau == 0:
                                bulk_mm(k + 2, 0)
                                bulk_mm(k + 2, 2)
                            elif tau == 1:
                                bulk_mm(k + 2, 4)
                                bulk_mm(k + 2, 6)
                            elif tau == 2:
                                bulk_mm(k + 2, 1)
                                bulk_mm(k + 2, 3)
                            else:
                                bulk_mm(k + 2, 5)
                                bulk_mm(k + 2, 7)

            # ==== phase B+C: Y = exp(em + bc) from the in-A emission store,
            # ==== interleaved with the CRF partition chains (exp and ln
            # ==== share the natural_log_exp activation table). Only blocks
            # ==== 0 and 31 (which need the very last LSTM iters) run their
            # ==== matmuls here.
            with tc.tile_pool(name="psB", bufs=2, space="PSUM") as pb, \
                 tc.tile_pool(name="psC", bufs=2, space="PSUM") as pc, \
                 tc.tile_pool(name="psC2", bufs=1, space="PSUM") as pc2:

                def emit_block(blk):
                    t0 = blk * EBLK
                    em_new = pb.tile([NB, EBLK * BC], F32, tag="em_ps")
                    for part in range(4):
                        th0_ = t0 + part * HBq
                        sl = slice(part * HBq * BC, (part + 1) * HBq * BC)
                        nc.tensor.matmul(
                            em_new[:, sl], wc[0:I, 0, :],
                            h_all[0:I, 0,
                                  (th0_ + 1) * BC:(th0_ + 1 + HBq) * BC],
                            start=True, stop=False, skip_group_check=True,
                        )
                        nc.tensor.matmul(
                            em_new[:, sl].rearrange("p (t b) -> p t b", b=BC),
                            wc[0:I, 1, :],
                            hb_ap_a[:, SL - th0_:SL - th0_ - HBq:-1, :],
                            start=False, stop=True, skip_group_check=True,
                        )
                    nc.scalar.activation(
                        Y[:, t0 * BC:(t0 + EBLK) * BC], em_new[:, :], EXP,
                        bias=bc_s[:, 0:1]
                    )

                def emit_exp(blk, q=None):
                    # exp over the bf16 pre-emissions stored during phase A.
                    # The bias is derived from CRF chain state (x*0 + bc) so
                    # the scheduler cannot hoist the EXP into phase A, where
                    # it would thrash the sigmoid/tanh activation table.
                    t0 = blk * EBLK
                    if q is None:
                        bias_ap = bc_s[:, 0:1]
                    else:
                        bcp = bcps[blk % 2]
                        nc.vector.scalar_tensor_tensor(
                            bcp[:, :], CH[:, q, 0, 0:1], 0.0, bc_s[:, 0:1],
                            op0=MULT, op1=ADD)
                        bias_ap = bcp[:, 0:1]
                    nc.scalar.activation(
                        Y[:, t0 * BC:(t0 + EBLK) * BC],
                        emstore[:, t0 * BC:(t0 + EBLK) * BC], EXP,
                        bias=bias_ap
                    )

                bcps = (bcp0, bcp1)
                emit_block(0)
                emit_block(31)
                # W0 = Y_0 * exp(T[BOS,:]) ; V = veb * Y_last
                # chain state in CH[parity, chain, :]: hop r reads parity
                # (r-1)%2, writes r%2 — no in-place WAR; the fwd and bwd hop
                # multiplies merge into ONE strided-AP tensor_tensor per r
                Yp = Y[0:NB, :].rearrange("p (t b) -> p t b", b=BC)
                nc.vector.tensor_scalar_mul(CH[:, 0, 0, :], Y[0:NB, 0:BC],
                                            etb[:, 0:1])
                nc.vector.tensor_mul(CH[:, 1, 1, :], veb[:, :],
                                     Y[0:NB, (SL - 1) * BC:SL * BC])

                # small phase-C PSUM tensors: one bank per chain so the fwd
                # and bwd renorm pipelines don't false-serialize on a bank
                crfF = pc2.tile([NB, 3 * BC], F32, tag="crfF")
                crfB = pc2.tile([NB, 2 * BC], F32, tag="crfB")
                rf_ps = crfF[:, 0:BC]
                sf_ps = crfF[0:1, BC:2 * BC]
                dot_ps = crfF[0:1, 2 * BC:3 * BC]
                rb_ps = crfB[:, 0:BC]
                sb_ps = crfB[0:1, BC:2 * BC]

                def renorm_snap(w_sb, s_ps):
                    # s = ones19^T @ w (PE, off the recurrence chain)
                    nc.tensor.matmul(s_ps, ones19[:, :], w_sb[:, :],
                                     skip_group_check=True)

                def renorm_mid(s_ps, rec, recb, r_ps, lg, acc):
                    nc.vector.reciprocal(rec[:, :], s_ps)
                    nc.vector.tensor_copy(recb[:, :], rec[:, :])
                    nc.tensor.matmul(r_ps, one1x19[:, :], recb[:, :],
                                     skip_group_check=True)
                    nc.scalar.activation(lg[:, :], s_ps, LOG)
                    nc.vector.tensor_add(acc[:, :], acc[:, :], lg[:, :])

                wb_prev = None
                HALF = SL // 2
                for r in range(HALF):
                    p, q = r % 2, (r + 1) % 2  # dst / src parity
                    ty = SL - 2 - r  # next Y column for backward chain
                    # just-in-time Y production (exp only): low block kk+1 at
                    # r=16kk+6, high block 30-kk at r=16kk+11 (blocks 0, 31
                    # are fully produced before the loop)
                    kk, jj = divmod(r, RENORM)
                    if kk <= 14 and jj == 6:
                        emit_exp(kk + 1, q)
                    elif kk <= 14 and jj == 11:
                        emit_exp(30 - kk, q)
                    wfb_ps = pc.tile([NB, 2, BC], F32, tag="wfb_ps")
                    # backward chain mm (always)
                    nc.tensor.matmul(wfb_ps[:, 1, :], est[:, :],
                                     CH[:, q, 1, :], skip_group_check=True)
                    # forward chain mm: t = r = 1..HALF-1
                    if r >= 1:
                        nc.tensor.matmul(wfb_ps[:, 0, :], esm[:, :],
                                         CH[:, q, 0, :], skip_group_check=True)
                    if r % RENORM == 0 and r >= RENORM:
                        renorm_snap(CH[:, q, 0, :], sf_ps)
                        renorm_snap(CH[:, q, 1, :], sb_ps)
                    # hop multiplies: one strided TT covers both chains
                    if 1 <= r < HALF - 1:
                        nc.vector.tensor_mul(
                            CH[:, p, :, :], wfb_ps[:, :, :],
                            Yp[:, r:ty + 1:(ty - r), :])
                    elif r == 0:
                        nc.vector.tensor_mul(
                            CH[:, p, 1, :], wfb_ps[:, 1, :],
                            Y[0:NB, ty * BC:(ty + 1) * BC])
                    else:  # r == HALF-1: forward hop only
                        nc.vector.tensor_mul(
                            CH[:, p, 0, :], wfb_ps[:, 0, :],
                            Y[0:NB, r * BC:(r + 1) * BC])
                    if r % RENORM == 0 and r >= RENORM:
                        renorm_mid(sf_ps, rec_f, rec_fb, rf_ps, lg_f, acc_f)
                        renorm_mid(sb_ps, rec_b, rec_bb, rb_ps, lg_b, acc_b)
                    if r >= RENORM + 3 and (r - 3) % RENORM == 0:
                        nc.vector.tensor_mul(CH[:, p, 0, :], CH[:, p, 0, :],
                                             rf_ps)
                        nc.vector.tensor_mul(CH[:, p, 1, :], CH[:, p, 1, :],
                                             rb_ps)
                    wb_prev = wfb_ps

                # meet at t=HALF-1: P2 = Wf_{HALF-1} * beta_{HALF-1}
                nc.vector.tensor_mul(P2[:, :], CH[:, (HALF - 1) % 2, 0, :],
                                     wb_prev[:, 1, :])
                nc.tensor.matmul(dot_ps, ones19f[:, :], P2[:, :],
                                 skip_group_check=True)
                nc.scalar.activation(res_s[0:1, :], dot_ps, LOG)

            nc.sync.dma_start(out=y_out[:], in_=Y[:, :])
            nc.sync.dma_start(out=res_out[0:1], in_=res_s[0:1, :])
            nc.sync.dma_start(out=res_out[1:2], in_=acc_f[:, :])
            nc.sync.dma_start(out=res_out[2:3], in_=acc_b[:, :])

    return nc


def _split_waits(nc):
    """Walrus codegen allows ~1 sync-wait on compute instrs; move excess
    waits onto injected same-engine Drain instructions (which allow many).

    Keep the wait most likely to be satisfied LAST inline on the compute
    instruction (a cross-engine producer), and drain the early-satisfied
    ones (same-engine program-order waits) first — a drain blocked on the
    critical producer adds ~70-90ns of serial queue decode vs an inline
    wait that fires as soon as the semaphore lands."""
    from concourse import mybir as mb

    def sem_engine(w):
        nm = getattr(w, 'ant_name', '') or ''
        return nm.split('_')[0]

    eng_name = {
        mb.EngineType.PE: 'PE', mb.EngineType.Activation: 'Activation',
        mb.EngineType.DVE: 'DVE', mb.EngineType.Pool: 'Pool',
        mb.EngineType.SP: 'SP',
    }
    n = 0
    for f in nc.m.functions:
        for blk in f.blocks:
            insts = blk.instructions
            new_list = []
            for ins in insts:
                si = ins.sync_info
                if si is not None and si.on_wait and len(si.on_wait) > 1:
                    waits = list(si.on_wait)
                    own = eng_name.get(ins.engine, '?')
                    cross = [w for w in waits if sem_engine(w) != own]
                    selfw = [w for w in waits if sem_engine(w) == own]
                    inline = [cross[-1]] if cross else [waits[-1]]
                    rest = [w for w in waits if w is not inline[0]]
                    # self-engine waits first (satisfied early), cross after
                    rest.sort(key=lambda w: 0 if sem_engine(w) == own else 1)
                    for w in rest:
                        d = mb.InstDrain(
                            name=f"{ins.name}-ws{n}", ins=[], outs=[])
                        d.engine = ins.engine
                        d.sync_info = mb.SyncInfo(on_wait=[w], on_update=[])
                        new_list.append(d)
                        n += 1
                    ins.sync_info = mb.SyncInfo(
                        on_wait=inline, on_update=list(si.on_update))
                new_list.append(ins)
            del insts[:]
            insts.extend(new_list)
    return n


def _prep_host(inputs):
    emb = np.asarray(inputs["emb"], np.float32)
    T = np.asarray(inputs["transitions"], np.float32)
    W1 = np.asarray(inputs["W1"], np.float32)
    b1 = np.asarray(inputs["b1"], np.float32)
    W2 = np.asarray(inputs["W2"], np.float32)
    b2 = np.asarray(inputs["b2"], np.float32)

    emb_pad = np.zeros((V, EPAD), np.float32)
    emb_pad[:, 0:I] = emb
    emb_pad[:, I] = 1.0  # bias-aug ones row

    # gate reorder: pytorch [i,f,g,o] -> ours [f,i,g,o]
    perm = np.concatenate([np.arange(I, 2 * I), np.arange(0, I),
                           np.arange(2 * I, 3 * I), np.arange(3 * I, 4 * I)])

    def pack_dir(Wih, Whh, bih, bhh):
        Wih, Whh = Wih[perm].copy(), Whh[perm].copy()
        bias = (bih + bhh)[perm].copy()
        # 2x the g-gate pre-acts: kernel computes tanh(g) as 2*sigma(2g)-1
        Wih[2 * I:3 * I] *= 2.0
        Whh[2 * I:3 * I] *= 2.0
        bias[2 * I:3 * I] *= 2.0
        wih = np.zeros((4, 128, 128), np.float32)  # [gamma, k, m]
        whh = np.zeros((4, 128, 128), np.float32)
        for g in range(4):
            wih[g, 0:I, 0:I] = Wih[g * I:(g + 1) * I].T
            wih[g, I, 0:I] = bias[g * I:(g + 1) * I]
            whh[g, 0:I, 0:I] = Whh[g * I:(g + 1) * I].T
        return wih, whh

    wih_f, whh_f = pack_dir(np.asarray(inputs["Wih_f"], np.float32),
                            np.asarray(inputs["Whh_f"], np.float32),
                            np.asarray(inputs["bih_f"], np.float32),
                            np.asarray(inputs["bhh_f"], np.float32))
    wih_b, whh_b = pack_dir(np.asarray(inputs["Wih_b"], np.float32),
                            np.asarray(inputs["Whh_b"], np.float32),
                            np.asarray(inputs["bih_b"], np.float32),
                            np.asarray(inputs["bhh_b"], np.float32))

    wih = np.zeros((128, GCH, 128), np.float32)
    whh = np.zeros((128, GCH, 128), np.float32)
    for g in range(4):
        wih[:, g * 2 + 0, :] = wih_f[g]
        wih[:, g * 2 + 1, :] = wih_b[g]
        whh[:, g * 2 + 0, :] = whh_f[g]
        whh[:, g * 2 + 1, :] = whh_b[g]

    Wc = W2 @ W1                      # [19, 200]
    bcv = W2 @ b1 + b2                # [19]
    wc = np.zeros((128, 2, NB), np.float32)
    wc[0:I, 0, :] = Wc[:, 0:I].T
    wc[0:I, 1, :] = Wc[:, I:2 * I].T

    c0 = float(np.log(np.sum(np.exp(bcv))))
    esm = np.exp(T - c0)
    est = esm.T.copy()
    etb = np.exp(T[BOS, :]).reshape(NB, 1)
    veb = np.broadcast_to(np.exp(T[:, EOS]).reshape(NB, 1), (NB, BC)).copy()

    bf = ml_dtypes.bfloat16
    common = {
        "_emb_bf": emb_pad.astype(bf),  # host-side only (xet pre-gather)
        "wih": wih.astype(bf),
        "whh": whh.astype(bf),
        "wc": wc.astype(bf),
        "bc": bcv.reshape(NB, 1).astype(np.float32),
        "esm": esm.astype(bf),
        "est": est.astype(bf),
        "etb": etb.astype(np.float32),
        "veb": veb.astype(np.float32),
        "ones19": np.ones((NB, 1), bf),
        "ones19f": np.ones((NB, 1), np.float32),
        "one1x19": np.ones((1, NB), bf),
    }
    return common, c0, bcv


def kernel(**inputs):
    x = np.asarray(inputs["x"]).reshape(B, S).astype(np.int64)
    target = np.asarray(inputs["target"]).reshape(B, S).astype(np.int64)
    T = np.asarray(inputs["transitions"], np.float32)

    common, c0, bcv = _prep_host(inputs)

    emb_bf = common.pop("_emb_bf")
    in_maps = []
    for c in range(NCORES):
        xs = x[c * BC:(c + 1) * BC]  # [BC, S]
        # fwd token order: col t*BC + b  -> x[b, t]; host pre-gathers the
        # (bias-augmented) embeddings pre-transposed: xet[:, t*BC+b]
        idx_fwd = xs.T.reshape(-1)
        xet = np.ascontiguousarray(emb_bf[idx_fwd].T)
        in_maps.append({**common, "xet": xet})

    if "nc" not in _CACHE:
        nc0 = _build_nc()
        _split_waits(nc0)
        mybir.codegen_inst_isa_subclasses(nc0)
        _CACHE["nc"] = nc0
    nc = _CACHE["nc"]
    _CACHE["in_maps"] = in_maps

    results = run_bass_kernel_spmd(nc, in_maps, list(range(NCORES))).results

    # host combine
    t_sc = (T[target[:, :-1], target[:, 1:]].sum(1)
            + T[BOS, target[:, 0]] + T[target[:, -1], EOS])  # [B]

    losses = np.zeros(B, np.float64)
    for c in range(NCORES):
        yv = np.asarray(results[c]["y_out"], ml_dtypes.bfloat16).astype(np.float32)
        res = np.asarray(results[c]["res"], np.float32)
        logY = np.log(yv).reshape(NB, S, BC)  # log Y = em + bc - c0... (em+bc)
        tg = target[c * BC:(c + 1) * BC]      # [BC, S]
        bi = np.arange(BC)
        e_sc = np.zeros(BC, np.float64)
        for t in range(S):
            e_sc += logY[tg[:, t], t, bi]
        partition = res[0] + res[1] + res[2] + (S - 1) * c0
        losses[c * BC:(c + 1) * BC] = (
            e_sc + t_sc[c * BC:(c + 1) * BC] - partition
        )
    return np.float32(-losses.mean())



# revision 46
# speedup vs baseline: 1.1265x; 1.0137x over previous
"""BiLSTM+CRF NLL loss kernel for 8 Trainium2 NeuronCores (v3).

Sharding: data-parallel on batch (32 sequences per core). Each core runs the
full BiLSTM + emission + CRF forward/backward partition recurrences for its
shard; host combines per-core partials into the scalar loss.

v3 vs v2 (trace: 2551ns/step chain = mm,σ,tanh_g,mul,add,tanh_c,mul_h):
- tanh(g) removed from the serial ACT chain: g pre-acts are scaled 2x on the
  host so one sigmoid instruction covers f,i,g (tanh(g) = 2σ(2g)-1), and the
  cell update becomes c = f*c + 2(σ_g-0.5)*i via fused scalar_tensor_tensor
- cell/gate DVE pipeline in bf16 (2x DVE rate), cell updated in place in a
  fixed tile (no cross-engine WAR)
- CRF fwd/bwd hop multiplies merged into one strided-AP tensor_tensor
"""

import numpy as np
import ml_dtypes

import concourse.bass as bass
import concourse.tile as tile
from concourse import mybir
from concourse.bass_utils import run_bass_kernel_spmd

F32 = mybir.dt.float32
BF16 = mybir.dt.bfloat16

B, S, V, I, NB = 256, 512, 30000, 100, 19
BOS, EOS = 17, 18
NCORES = 8
BC = B // NCORES          # 32 sequences per core
NT = BC * S               # 16384 tokens per core
KP = I + 1                # 101: embedding dims + ones row (bias aug)
EPAD = 128                # padded embedding row length
RENORM = 16               # CRF renorm interval
TBLK = 4                  # steps per PSUM gate block
GCH = 8                   # gate chunks: (gamma in [g,f,i,o]) x (dir in [f,b])

_CACHE = {}


def _build_nc(s_len=S):
    SL = s_len
    NTL = BC * SL
    NBLK = SL // TBLK
    NCHUNK = NTL // 128

    nc = bass.Bass()

    # ---- dram I/O ----
    # xet = host-pre-gathered, pre-transposed embeddings (bias-aug row
    # included): the on-device indirect gather + transpose machinery was the
    # phase A startup bottleneck (random 256B-row HBM gathers at ~3us/chunk)
    xet_d = nc.dram_tensor("xet", [128, NTL], BF16, kind="ExternalInput")
    wih_d = nc.dram_tensor("wih", [128, GCH, 128], BF16, kind="ExternalInput")
    whh_d = nc.dram_tensor("whh", [128, GCH, 128], BF16, kind="ExternalInput")
    wc_d = nc.dram_tensor("wc", [128, 2, NB], BF16, kind="ExternalInput")
    bc_d = nc.dram_tensor("bc", [NB, 1], F32, kind="ExternalInput")
    esm_d = nc.dram_tensor("esm", [NB, NB], BF16, kind="ExternalInput")
    est_d = nc.dram_tensor("est", [NB, NB], BF16, kind="ExternalInput")
    etb_d = nc.dram_tensor("etb", [NB, 1], F32, kind="ExternalInput")
    veb_d = nc.dram_tensor("veb", [NB, BC], F32, kind="ExternalInput")
    ones19_d = nc.dram_tensor("ones19", [NB, 1], BF16, kind="ExternalInput")
    ones19f_d = nc.dram_tensor("ones19f", [NB, 1], F32, kind="ExternalInput")
    one1x19_d = nc.dram_tensor("one1x19", [1, NB], BF16, kind="ExternalInput")

    y_out = nc.dram_tensor("y_out", [NB, NTL], BF16, kind="ExternalOutput")
    res_out = nc.dram_tensor("res", [4, BC], F32, kind="ExternalOutput")

    SIG = mybir.ActivationFunctionType.Sigmoid
    TANH = mybir.ActivationFunctionType.Tanh
    EXP = mybir.ActivationFunctionType.Exp
    LOG = mybir.ActivationFunctionType.Ln

    with tile.TileContext(nc) as tc:
        with tc.tile_pool(name="big", bufs=1) as bp:
            xeT_f = bp.tile([128, NTL], BF16, tag="xeT_f")
            emstore = bp.tile([NB, NTL], BF16, tag="emstore")
            # h storage: col (t+1)*32 = h after step t; col 0 = h(-1)=0
            h_all = bp.tile([128, 2, NTL + BC], BF16, tag="h_all")
            Y = bp.tile([NB, NTL], BF16, tag="Y")
            wih = bp.tile([128, GCH, 128], BF16, tag="wih")
            whh = bp.tile([128, GCH, 128], BF16, tag="whh")
            wc = bp.tile([128, 2, NB], BF16, tag="wc")
            bc_s = bp.tile([NB, 1], F32, tag="bc_s")
            esm = bp.tile([NB, NB], BF16, tag="esm")
            est = bp.tile([NB, NB], BF16, tag="est")
            etb = bp.tile([NB, 1], F32, tag="etb")
            veb = bp.tile([NB, BC], F32, tag="veb")
            ones19 = bp.tile([NB, 1], BF16, tag="ones19")
            ones19f = bp.tile([NB, 1], F32, tag="ones19f")
            one1x19 = bp.tile([1, NB], BF16, tag="one1x19")
            gates_s0 = bp.tile([128, GCH, BC], BF16, tag="gates_s0")
            gates_s1 = bp.tile([128, GCH, BC], BF16, tag="gates_s1")
            cellc = bp.tile([128, 2, BC], BF16, tag="cellc")  # c, in-place
            th0 = bp.tile([128, 2, BC], BF16, tag="th0")
            th1 = bp.tile([128, 2, BC], BF16, tag="th1")
            u_f = bp.tile([128, 2, BC], BF16, tag="u_f")
            u_i = bp.tile([128, 2, BC], BF16, tag="u_i")
            # CRF chain state: [parity, chain(0=fwd,1=bwd), BC]
            CH = bp.tile([NB, 2, 2, BC], BF16, tag="CH")
            P2 = bp.tile([NB, BC], F32, tag="P2")
            acc_f = bp.tile([1, BC], F32, tag="acc_f")
            acc_b = bp.tile([1, BC], F32, tag="acc_b")
            bcp0 = bp.tile([NB, 1], F32, tag="bcp0")
            bcp1 = bp.tile([NB, 1], F32, tag="bcp1")
            rec_f = bp.tile([1, BC], F32, tag="rec_f")
            rec_fb = bp.tile([1, BC], BF16, tag="rec_fb")
            rec_b = bp.tile([1, BC], F32, tag="rec_b")
            rec_bb = bp.tile([1, BC], BF16, tag="rec_bb")
            lg_f = bp.tile([1, BC], F32, tag="lg_f")
            lg_b = bp.tile([1, BC], F32, tag="lg_b")
            res_s = bp.tile([4, BC], F32, tag="res_s")

            # ---- loads: early xet ranges first (blocks 0.. and ..127),
            # ---- then weights, then the xet middle ----
            C16 = 16 * 128
            nc.sync.dma_start(out=xeT_f[:, 0:C16], in_=xet_d[:, 0:C16])
            nc.sync.dma_start(out=xeT_f[:, NTL - C16:NTL],
                              in_=xet_d[:, NTL - C16:NTL])
            nc.sync.dma_start(out=wih[:, :, :], in_=wih_d[:])
            nc.sync.dma_start(out=whh[:, :, :], in_=whh_d[:])
            nc.sync.dma_start(out=wc[:, :, :], in_=wc_d[:])
            nc.sync.dma_start(out=bc_s[:, :], in_=bc_d[:])
            nc.sync.dma_start(out=esm[:, :], in_=esm_d[:])
            nc.sync.dma_start(out=est[:, :], in_=est_d[:])
            nc.sync.dma_start(out=etb[:, :], in_=etb_d[:])
            nc.sync.dma_start(out=veb[:, :], in_=veb_d[:])
            nc.sync.dma_start(out=ones19[:, :], in_=ones19_d[:])
            nc.sync.dma_start(out=ones19f[:, :], in_=ones19f_d[:])
            nc.sync.dma_start(out=one1x19[:, :], in_=one1x19_d[:])
            # xet middle: two big DMAs, landing well before consumption
            nc.sync.dma_start(out=xeT_f[:, C16:NTL // 2],
                              in_=xet_d[:, C16:NTL // 2])
            nc.sync.dma_start(out=xeT_f[:, NTL // 2:NTL - C16],
                              in_=xet_d[:, NTL // 2:NTL - C16])

            nc.vector.memset(acc_f[:, :], 0.0)
            nc.vector.memset(acc_b[:, :], 0.0)

            # =========== phase A: both LSTMs, lockstep ==========
            with tc.tile_pool(name="psE", bufs=2, space="PSUM") as pe_pool, \
                 tc.tile_pool(name="psA", bufs=1, space="PSUM") as pa:
                GA = pa.tile([128, GCH, TBLK, BC], F32, tag="GA")
                GB = pa.tile([128, GCH, TBLK, BC], F32, tag="GB")
                gbuf = (GA, GB)

                xe3 = xeT_f[0:KP, :].rearrange("p (t b) -> p t b", b=BC)

                def bulk_mm(k, c):
                    G = gbuf[k % 2]
                    if c % 2 == 0:
                        rhs = xe3[:, k * TBLK:(k + 1) * TBLK, :]
                    else:
                        hi = SL - 1 - k * TBLK
                        rhs = (xe3[:, hi:hi - TBLK:-1, :]
                               if hi - TBLK >= 0 else xe3[:, hi::-1, :])
                    nc.tensor.matmul(
                        G[:, c, :, :], wih[0:KP, c, :], rhs,
                        start=True, stop=False, skip_group_check=True,
                    )

                MULT = mybir.AluOpType.mult
                ADD = mybir.AluOpType.add
                SUB = mybir.AluOpType.subtract

                def step(t):
                    G = gbuf[(t // TBLK) % 2]
                    tau = t % TBLK
                    rd = t * BC
                    gs = gates_s0 if t % 2 == 0 else gates_s1
                    th = th0 if t % 2 == 0 else th1
                    if t > 0:
                        # recurrent matmuls: f,i,g first, then o
                        for c in (0, 1, 2, 3, 4, 5):
                            d = c % 2
                            nc.tensor.matmul(
                                G[:, c, tau, :], whh[0:I, c, :],
                                h_all[0:I, d, rd:rd + BC],
                                start=False, stop=True, skip_group_check=True,
                            )
                    # one sigmoid covers f,i,g (g pre-acts carry a 2x host
                    # scale, so sigma here encodes tanh(g) = 2*sigma(2g)-1)
                    nc.scalar.activation(gs[:, 0:6, :], G[:, 0:6, tau, :], SIG)
                    if t > 0:
                        for c in (6, 7):
                            d = c % 2
                            nc.tensor.matmul(
                                G[:, c, tau, :], whh[0:I, c, :],
                                h_all[0:I, d, rd:rd + BC],
                                start=False, stop=True, skip_group_check=True,
                            )
                    # sigmoid(o) off the critical path
                    nc.scalar.activation(gs[:, 6:8, :], G[:, 6:8, tau, :], SIG)
                    # cellc tracks the HALF-cell d = c/2, making the cell
                    # update end in a plain add: d = f*d + (sigma_g-0.5)*i
                    # [= f*c/2 + tanh(g)/2*i]; tanh(c) = tanh(2d) via scale
                    nc.vector.scalar_tensor_tensor(
                        u_i[:, :, :], gs[:, 4:6, :], 0.5, gs[:, 2:4, :],
                        op0=SUB, op1=MULT)
                    if t > 0:
                        nc.vector.tensor_mul(
                            u_f[:, :, :], gs[:, 0:2, :], cellc[:, :, :])
                        nc.vector.tensor_add(
                            cellc[:, :, :], u_f[:, :, :], u_i[:, :, :])
                    else:
                        nc.vector.tensor_copy(cellc[:, :, :], u_i[:, :, :])
                    nc.scalar.activation(th[:, :, :], cellc[:, :, :], TANH,
                                         scale=2.0)
                    wr = (t + 1) * BC
                    nc.vector.tensor_mul(
                        h_all[:, :, wr:wr + BC], gs[:, 6:8, :], th[:, :, :]
                    )

                for c in range(GCH):
                    bulk_mm(0, c)
                for c in range(GCH):
                    bulk_mm(1, c)

                # in-A emission production: emission matmuls for the middle
                # blocks run here (h for block j is complete from iter
                # max(16j+15, 496-16j); PE has slack), storing pre-exp
                # emissions to SBUF in bf16. Phase C then only runs the EXPs
                # (no activation-table thrash: exp stays out of phase A).
                EBLK = 16
                HBq = 4  # tokens per emission quarter-matmul
                hb_ap_a = h_all[0:I, 1, :].rearrange("p (t b) -> p t b", b=BC)
                ema_tiles = {}

                def ema_quarter(blk, part):
                    t0 = blk * EBLK
                    if blk not in ema_tiles:
                        ema_new = pe_pool.tile([NB, EBLK * BC], F32,
                                               tag="ema_ps")
                        ema_tiles[blk] = ema_new
                    em_ps = ema_tiles[blk]
                    th0_ = t0 + part * HBq
                    sl = slice(part * HBq * BC, (part + 1) * HBq * BC)
                    # hf for token t lives at col (t+1)*BC
                    nc.tensor.matmul(
                        em_ps[:, sl], wc[0:I, 0, :],
                        h_all[0:I, 0, (th0_ + 1) * BC:(th0_ + 1 + HBq) * BC],
                        start=True, stop=False, skip_group_check=True,
                    )
                    # hb for token t lives at round (SL-1-t): col (SL-t)*BC
                    nc.tensor.matmul(
                        em_ps[:, sl].rearrange("p (t b) -> p t b", b=BC),
                        wc[0:I, 1, :],
                        hb_ap_a[:, SL - th0_:SL - th0_ - HBq:-1, :],
                        start=False, stop=True, skip_group_check=True,
                    )

                def ema_cast(blk, half):
                    # halves bound the DVE head-of-line delay to ~370ns
                    t0 = blk * EBLK
                    em_ps = ema_tiles[blk]
                    HC = EBLK * BC // 2
                    nc.vector.tensor_copy(
                        emstore[:, t0 * BC + half * HC:t0 * BC + (half + 1) * HC],
                        em_ps[:, half * HC:(half + 1) * HC])
                    if half == 1:
                        ema_tiles.pop(blk)

                def ema_slot(t):
                    # window k covers iters [264+16k, 280+16k) and produces
                    # blocks lo=15-k and hi=16+k. Per-part readiness (h_f[t]
                    # at iter t, h_b[t] at iter 511-t): lo part p ready at
                    # iter 271+16k-4p (reverse order), hi part p at 259+16k+4p
                    if not (264 <= t < 504):
                        return
                    kwin, loc = divmod(t - 264, 16)
                    lo, hi = 15 - kwin, 16 + kwin
                    if loc == 0:
                        ema_quarter(lo, 3)
                    elif loc == 1:
                        ema_quarter(lo, 2)
                    elif loc == 2:
                        ema_quarter(hi, 0)
                    elif loc == 5:
                        ema_quarter(lo, 1)
                    elif loc == 4:
                        ema_quarter(hi, 1)
                    elif loc == 6:
                        ema_quarter(hi, 2)
                    elif loc == 8:
                        ema_quarter(lo, 0)
                    elif loc == 9:
                        ema_quarter(hi, 3)
                    elif loc == 10:
                        ema_cast(lo, 0)
                    elif loc == 11:
                        ema_cast(lo, 1)
                    elif loc == 12:
                        ema_cast(hi, 0)
                    elif loc == 13:
                        ema_cast(hi, 1)

                # chunk c serves fwd block c and bwd block NCHUNK-1-c, so every
                # chunk must be resident before block NBLK//2. DMA the chunk
                for k in range(NBLK):
                    for tau in range(TBLK):
                        step(k * TBLK + tau)
                        ema_slot(k * TBLK + tau)
                        # spread bulk (x-part) matmuls across the block
                        if k + 2 < NBLK:
                            if tau == 0:
                                bulk_mm(k + 2, 0)
                                bulk_mm(k + 2, 2)
                            elif tau == 1:
                                bulk_mm(k + 2, 4)
                                bulk_mm(k + 2, 6)
                            elif tau == 2:
                                bulk_mm(k + 2, 1)
                                bulk_mm(k + 2, 3)
                            else:
                                bulk_mm(k + 2, 5)
                                bulk_mm(k + 2, 7)

            # ==== phase B+C: Y = exp(em + bc) from the in-A emission store,
            # ==== interleaved with the CRF partition chains (exp and ln
            # ==== share the natural_log_exp activation table). Only blocks
            # ==== 0 and 31 (which need the very last LSTM iters) run their
            # ==== matmuls here.
            with tc.tile_pool(name="psB", bufs=2, space="PSUM") as pb, \
                 tc.tile_pool(name="psC", bufs=2, space="PSUM") as pc, \
                 tc.tile_pool(name="psC2", bufs=1, space="PSUM") as pc2:

                def emit_block(blk):
                    t0 = blk * EBLK
                    em_new = pb.tile([NB, EBLK * BC], F32, tag="em_ps")
                    for part in range(4):
                        th0_ = t0 + part * HBq
                        sl = slice(part * HBq * BC, (part + 1) * HBq * BC)
                        nc.tensor.matmul(
                            em_new[:, sl], wc[0:I, 0, :],
                            h_all[0:I, 0,
                                  (th0_ + 1) * BC:(th0_ + 1 + HBq) * BC],
                            start=True, stop=False, skip_group_check=True,
                        )
                        nc.tensor.matmul(
                            em_new[:, sl].rearrange("p (t b) -> p t b", b=BC),
                            wc[0:I, 1, :],
                            hb_ap_a[:, SL - th0_:SL - th0_ - HBq:-1, :],
                            start=False, stop=True, skip_group_check=True,
                        )
                    nc.scalar.activation(
                        Y[:, t0 * BC:(t0 + EBLK) * BC], em_new[:, :], EXP,
                        bias=bc_s[:, 0:1]
                    )

                def emit_exp(blk, q=None):
                    # exp over the bf16 pre-emissions stored during phase A.
                    # The bias is derived from CRF chain state (x*0 + bc) so
                    # the scheduler cannot hoist the EXP into phase A, where
                    # it would thrash the sigmoid/tanh activation table.
                    t0 = blk * EBLK
                    if q is None:
                        bias_ap = bc_s[:, 0:1]
                    else:
                        bcp = bcps[blk % 2]
                        nc.vector.scalar_tensor_tensor(
                            bcp[:, :], CH[:, q, 0, 0:1], 0.0, bc_s[:, 0:1],
                            op0=MULT, op1=ADD)
                        bias_ap = bcp[:, 0:1]
                    nc.scalar.activation(
                        Y[:, t0 * BC:(t0 + EBLK) * BC],
                        emstore[:, t0 * BC:(t0 + EBLK) * BC], EXP,
                        bias=bias_ap
                    )

                bcps = (bcp0, bcp1)
                emit_block(0)
                emit_block(31)
                # W0 = Y_0 * exp(T[BOS,:]) ; V = veb * Y_last
                # chain state in CH[parity, chain, :]: hop r reads parity
                # (r-1)%2, writes r%2 — no in-place WAR; the fwd and bwd hop
                # multiplies merge into ONE strided-AP tensor_tensor per r
                Yp = Y[0:NB, :].rearrange("p (t b) -> p t b", b=BC)
                nc.vector.tensor_scalar_mul(CH[:, 0, 0, :], Y[0:NB, 0:BC],
                                            etb[:, 0:1])
                nc.vector.tensor_mul(CH[:, 1, 1, :], veb[:, :],
                                     Y[0:NB, (SL - 1) * BC:SL * BC])

                # small phase-C PSUM tensors: one bank per chain so the fwd
                # and bwd renorm pipelines don't false-serialize on a bank
                crfF = pc2.tile([NB, 3 * BC], F32, tag="crfF")
                crfB = pc2.tile([NB, 2 * BC], F32, tag="crfB")
                rf_ps = crfF[:, 0:BC]
                sf_ps = crfF[0:1, BC:2 * BC]
                dot_ps = crfF[0:1, 2 * BC:3 * BC]
                rb_ps = crfB[:, 0:BC]
                sb_ps = crfB[0:1, BC:2 * BC]

                def renorm_snap(w_sb, s_ps):
                    # s = ones19^T @ w (PE, off the recurrence chain)
                    nc.tensor.matmul(s_ps, ones19[:, :], w_sb[:, :],
                                     skip_group_check=True)

                def renorm_mid(s_ps, rec, recb, r_ps, lg, acc):
                    nc.vector.reciprocal(rec[:, :], s_ps)
                    nc.vector.tensor_copy(recb[:, :], rec[:, :])
                    nc.tensor.matmul(r_ps, one1x19[:, :], recb[:, :],
                                     skip_group_check=True)
                    nc.scalar.activation(lg[:, :], s_ps, LOG)
                    nc.vector.tensor_add(acc[:, :], acc[:, :], lg[:, :])

                wb_prev = None
                HALF = SL // 2
                for r in range(HALF):
                    p, q = r % 2, (r + 1) % 2  # dst / src parity
                    ty = SL - 2 - r  # next Y column for backward chain
                    # just-in-time Y production (exp only): low block kk+1 at
                    # r=16kk+6, high block 30-kk at r=16kk+11 (blocks 0, 31
                    # are fully produced before the loop)
                    kk, jj = divmod(r, RENORM)
                    if kk <= 14 and jj == 6:
                        emit_exp(kk + 1, q)
                    elif kk <= 14 and jj == 11:
                        emit_exp(30 - kk, q)
                    # stream y_out back as each quarter of Y finalizes, so
                    # the ~12us strided DMA overlaps the chain instead of
                    # trailing the kernel
                    QY = 8 * EBLK * BC
                    if r == 104:
                        nc.sync.dma_start(out=y_out[:, 0:QY],
                                          in_=Y[:, 0:QY])
                    elif r == 110:
                        nc.sync.dma_start(out=y_out[:, 3 * QY:NTL],
                                          in_=Y[:, 3 * QY:NTL])
                    elif r == 232:
                        nc.sync.dma_start(out=y_out[:, QY:2 * QY],
                                          in_=Y[:, QY:2 * QY])
                    elif r == 238:
                        nc.sync.dma_start(out=y_out[:, 2 * QY:3 * QY],
                                          in_=Y[:, 2 * QY:3 * QY])
                    wfb_ps = pc.tile([NB, 2, BC], F32, tag="wfb_ps")
                    # backward chain mm (always)
                    nc.tensor.matmul(wfb_ps[:, 1, :], est[:, :],
                                     CH[:, q, 1, :], skip_group_check=True)
                    # forward chain mm: t = r = 1..HALF-1
                    if r >= 1:
                        nc.tensor.matmul(wfb_ps[:, 0, :], esm[:, :],
                                         CH[:, q, 0, :], skip_group_check=True)
                    if r % RENORM == 0 and r >= RENORM:
                        renorm_snap(CH[:, q, 0, :], sf_ps)
                        renorm_snap(CH[:, q, 1, :], sb_ps)
                    # hop multiplies: one strided TT covers both chains
                    if 1 <= r < HALF - 1:
                        nc.vector.tensor_mul(
                            CH[:, p, :, :], wfb_ps[:, :, :],
                            Yp[:, r:ty + 1:(ty - r), :])
                    elif r == 0:
                        nc.vector.tensor_mul(
                            CH[:, p, 1, :], wfb_ps[:, 1, :],
                            Y[0:NB, ty * BC:(ty + 1) * BC])
                    else:  # r == HALF-1: forward hop only
                        nc.vector.tensor_mul(
                            CH[:, p, 0, :], wfb_ps[:, 0, :],
                            Y[0:NB, r * BC:(r + 1) * BC])
                    if r % RENORM == 0 and r >= RENORM:
                        renorm_mid(sf_ps, rec_f, rec_fb, rf_ps, lg_f, acc_f)
                        renorm_mid(sb_ps, rec_b, rec_bb, rb_ps, lg_b, acc_b)
                    if r >= RENORM + 3 and (r - 3) % RENORM == 0:
                        nc.vector.tensor_mul(CH[:, p, 0, :], CH[:, p, 0, :],
                                             rf_ps)
                        nc.vector.tensor_mul(CH[:, p, 1, :], CH[:, p, 1, :],
                                             rb_ps)
                    wb_prev = wfb_ps

                # meet at t=HALF-1: P2 = Wf_{HALF-1} * beta_{HALF-1}
                nc.vector.tensor_mul(P2[:, :], CH[:, (HALF - 1) % 2, 0, :],
                                     wb_prev[:, 1, :])
                nc.tensor.matmul(dot_ps, ones19f[:, :], P2[:, :],
                                 skip_group_check=True)
                nc.scalar.activation(res_s[0:1, :], dot_ps, LOG)

            nc.sync.dma_start(out=res_out[0:1], in_=res_s[0:1, :])
            nc.sync.dma_start(out=res_out[1:2], in_=acc_f[:, :])
            nc.sync.dma_start(out=res_out[2:3], in_=acc_b[:, :])

    return nc


def _split_waits(nc):
    """Walrus codegen allows ~1 sync-wait on compute instrs; move excess
    waits onto injected same-engine Drain instructions (which allow many).

    Keep the wait most likely to be satisfied LAST inline on the compute
    instruction (a cross-engine producer), and drain the early-satisfied
    ones (same-engine program-order waits) first — a drain blocked on the
    critical producer adds ~70-90ns of serial queue decode vs an inline
    wait that fires as soon as the semaphore lands."""
    from concourse import mybir as mb

    def sem_engine(w):
        nm = getattr(w, 'ant_name', '') or ''
        return nm.split('_')[0]

    eng_name = {
        mb.EngineType.PE: 'PE', mb.EngineType.Activation: 'Activation',
        mb.EngineType.DVE: 'DVE', mb.EngineType.Pool: 'Pool',
        mb.EngineType.SP: 'SP',
    }
    n = 0
    for f in nc.m.functions:
        for blk in f.blocks:
            insts = blk.instructions
            new_list = []
            for ins in insts:
                si = ins.sync_info
                if si is not None and si.on_wait and len(si.on_wait) > 1:
                    waits = list(si.on_wait)
                    own = eng_name.get(ins.engine, '?')
                    cross = [w for w in waits if sem_engine(w) != own]
                    selfw = [w for w in waits if sem_engine(w) == own]
                    inline = [cross[-1]] if cross else [waits[-1]]
                    rest = [w for w in waits if w is not inline[0]]
                    # self-engine waits first (satisfied early), cross after
                    rest.sort(key=lambda w: 0 if sem_engine(w) == own else 1)
                    for w in rest:
                        d = mb.InstDrain(
                            name=f"{ins.name}-ws{n}", ins=[], outs=[])
                        d.engine = ins.engine
                        d.sync_info = mb.SyncInfo(on_wait=[w], on_update=[])
                        new_list.append(d)
                        n += 1
                    ins.sync_info = mb.SyncInfo(
                        on_wait=inline, on_update=list(si.on_update))
                new_list.append(ins)
            del insts[:]
            insts.extend(new_list)
    return n


def _prep_host(inputs):
    emb = np.asarray(inputs["emb"], np.float32)
    T = np.asarray(inputs["transitions"], np.float32)
    W1 = np.asarray(inputs["W1"], np.float32)
    b1 = np.asarray(inputs["b1"], np.float32)
    W2 = np.asarray(inputs["W2"], np.float32)
    b2 = np.asarray(inputs["b2"], np.float32)

    emb_pad = np.zeros((V, EPAD), np.float32)
    emb_pad[:, 0:I] = emb
    emb_pad[:, I] = 1.0  # bias-aug ones row

    # gate reorder: pytorch [i,f,g,o] -> ours [f,i,g,o]
    perm = np.concatenate([np.arange(I, 2 * I), np.arange(0, I),
                           np.arange(2 * I, 3 * I), np.arange(3 * I, 4 * I)])

    def pack_dir(Wih, Whh, bih, bhh):
        Wih, Whh = Wih[perm].copy(), Whh[perm].copy()
        bias = (bih + bhh)[perm].copy()
        # 2x the g-gate pre-acts: kernel computes tanh(g) as 2*sigma(2g)-1
        Wih[2 * I:3 * I] *= 2.0
        Whh[2 * I:3 * I] *= 2.0
        bias[2 * I:3 * I] *= 2.0
        wih = np.zeros((4, 128, 128), np.float32)  # [gamma, k, m]
        whh = np.zeros((4, 128, 128), np.float32)
        for g in range(4):
            wih[g, 0:I, 0:I] = Wih[g * I:(g + 1) * I].T
            wih[g, I, 0:I] = bias[g * I:(g + 1) * I]
            whh[g, 0:I, 0:I] = Whh[g * I:(g + 1) * I].T
        return wih, whh

    wih_f, whh_f = pack_dir(np.asarray(inputs["Wih_f"], np.float32),
                            np.asarray(inputs["Whh_f"], np.float32),
                            np.asarray(inputs["bih_f"], np.float32),
                            np.asarray(inputs["bhh_f"], np.float32))
    wih_b, whh_b = pack_dir(np.asarray(inputs["Wih_b"], np.float32),
                            np.asarray(inputs["Whh_b"], np.float32),
                            np.asarray(inputs["bih_b"], np.float32),
                            np.asarray(inputs["bhh_b"], np.float32))

    wih = np.zeros((128, GCH, 128), np.float32)
    whh = np.zeros((128, GCH, 128), np.float32)
    for g in range(4):
        wih[:, g * 2 + 0, :] = wih_f[g]
        wih[:, g * 2 + 1, :] = wih_b[g]
        whh[:, g * 2 + 0, :] = whh_f[g]
        whh[:, g * 2 + 1, :] = whh_b[g]

    Wc = W2 @ W1                      # [19, 200]
    bcv = W2 @ b1 + b2                # [19]
    wc = np.zeros((128, 2, NB), np.float32)
    wc[0:I, 0, :] = Wc[:, 0:I].T
    wc[0:I, 1, :] = Wc[:, I:2 * I].T

    c0 = float(np.log(np.sum(np.exp(bcv))))
    esm = np.exp(T - c0)
    est = esm.T.copy()
    etb = np.exp(T[BOS, :]).reshape(NB, 1)
    veb = np.broadcast_to(np.exp(T[:, EOS]).reshape(NB, 1), (NB, BC)).copy()

    bf = ml_dtypes.bfloat16
    common = {
        "_emb_bf": emb_pad.astype(bf),  # host-side only (xet pre-gather)
        "wih": wih.astype(bf),
        "whh": whh.astype(bf),
        "wc": wc.astype(bf),
        "bc": bcv.reshape(NB, 1).astype(np.float32),
        "esm": esm.astype(bf),
        "est": est.astype(bf),
        "etb": etb.astype(np.float32),
        "veb": veb.astype(np.float32),
        "ones19": np.ones((NB, 1), bf),
        "ones19f": np.ones((NB, 1), np.float32),
        "one1x19": np.ones((1, NB), bf),
    }
    return common, c0, bcv


def kernel(**inputs):
    x = np.asarray(inputs["x"]).reshape(B, S).astype(np.int64)
    target = np.asarray(inputs["target"]).reshape(B, S).astype(np.int64)
    T = np.asarray(inputs["transitions"], np.float32)

    common, c0, bcv = _prep_host(inputs)

    emb_bf = common.pop("_emb_bf")
    in_maps = []
    for c in range(NCORES):
        xs = x[c * BC:(c + 1) * BC]  # [BC, S]
        # fwd token order: col t*BC + b  -> x[b, t]; host pre-gathers the
        # (bias-augmented) embeddings pre-transposed: xet[:, t*BC+b]
        idx_fwd = xs.T.reshape(-1)
        xet = np.ascontiguousarray(emb_bf[idx_fwd].T)
        in_maps.append({**common, "xet": xet})

    if "nc" not in _CACHE:
        nc0 = _build_nc()
        _split_waits(nc0)
        mybir.codegen_inst_isa_subclasses(nc0)
        _CACHE["nc"] = nc0
    nc = _CACHE["nc"]
    _CACHE["in_maps"] = in_maps

    results = run_bass_kernel_spmd(nc, in_maps, list(range(NCORES))).results

    # host combine
    t_sc = (T[target[:, :-1], target[:, 1:]].sum(1)
            + T[BOS, target[:, 0]] + T[target[:, -1], EOS])  # [B]

    losses = np.zeros(B, np.float64)
    for c in range(NCORES):
        yv = np.asarray(results[c]["y_out"], ml_dtypes.bfloat16).astype(np.float32)
        res = np.asarray(results[c]["res"], np.float32)
        logY = np.log(yv).reshape(NB, S, BC)  # log Y = em + bc - c0... (em+bc)
        tg = target[c * BC:(c + 1) * BC]      # [BC, S]
        bi = np.arange(BC)
        e_sc = np.zeros(BC, np.float64)
        for t in range(S):
            e_sc += logY[tg[:, t], t, bi]
        partition = res[0] + res[1] + res[2] + (S - 1) * c0
        losses[c * BC:(c + 1) * BC] = (
            e_sc + t_sc[c * BC:(c + 1) * BC] - partition
        )
    return np.float32(-losses.mean())

